# revision 45
# baseline (speedup 1.0000x reference)
"""DMR induction routing kernel for Trainium2 (Bass/Tile), 8-core data-parallel.

Problem: nn_DMRInduction. Full inputs:
  m [128, 768], q [256, 768], W [768, 765], b [765] -> out [256, 765] fp32.

Sharding: Q=256 split 8 ways (32 queries/core); m, W, b replicated.

Per-core layouts:
  - hat_m        [I=128, C*D=765]   (I on partitions)  - hv weights / final hv rhs
  - hmT aug      [D+1=154, I] per c (D on partitions)  - num/mdv weights;
      row 153 holds -mean_c(m) so the num matmul computes the centered
      correlation numerator directly (sum_d xm*tq = sum_d m*tq - mum*colsum).
  - tq, v        [D, C*Q=160] as two tiles [128,160] + [34,160]
      (tqB row 32 carries colsum for the augmented num matmul and the
       yn2 correction; vB rows 25..33 stay zero so mdv stays uncentered).
  - routing state a, p, dsp [I=128, C*Q=160].
  - final hat_v  [Q=32, C*D=765] -> squash -> contiguous DMA out.
"""
import os
import sys

for _p in ("/opt/trn_rl_repo", "/root/.axon_site/_ro/trn_rl_repo"):
    if os.path.isdir(_p) and _p not in sys.path:
        sys.path.insert(0, _p)

import numpy as np
import concourse.bass as bass
import concourse.bacc as bacc
import concourse.mybir as mybir
import concourse.tile as tile
from concourse.bass_utils import run_bass_kernel_spmd

F32 = mybir.dt.float32
# Matmul input dtype. float32 is exact (final scale-relative err ~2e-5);
# float32r uses the fast PE path (1 cyc/row at N>=256 vs 4) and cuts the
# projection phase ~14us, at ~2.5e-4 scale-relative output error. The
# rest of the kernel is dependency-latency-bound, so the dtype only
# affects the projection matmuls. Default to exact.
DT = getattr(mybir.dt, os.environ.get("KERNEL_MM_DT", "float32r"))

NCORES = 8
I = 128         # memory capsules
C = 5           # capsule classes
D = 153         # dim per capsule
CD = C * D      # 765
K = 768         # input dim
KC = K // 128   # 6 contraction chunks
QL = 32         # queries per core
CQ = C * QL     # 160
NPAD = 768      # W padded to 768 cols so fp32r matmuls stream N>=256
EPS = 1e-8
AX = mybir.AxisListType.X
MUL = mybir.AluOpType.mult
ADD = mybir.AluOpType.add
SUB = mybir.AluOpType.subtract


def build(with_bias: bool, dbg: bool = False):
    nc = bacc.Bacc("TRN2", target_bir_lowering=False, debug=False)

    BF = mybir.dt.bfloat16
    # inputs arrive host-prearranged p-major so each load is one contiguous
    # descriptor per partition: x_pre[p, k, n] = x[k*128+p, n]
    mT_d = nc.dram_tensor("mTp", [128, KC * I], BF, kind="ExternalInput")
    qT_d = nc.dram_tensor("qTp", [128, KC * QL], BF, kind="ExternalInput")
    W_d = nc.dram_tensor("Wpp", [128, KC * NPAD], BF, kind="ExternalInput")
    b_d = nc.dram_tensor("b", [1, CD], F32, kind="ExternalInput")
    eye_d = nc.dram_tensor("eye", [128, 128], F32, kind="ExternalInput")
    out_d = nc.dram_tensor("out", [QL, CD], F32, kind="ExternalOutput")
    dbg_d = {}
    if dbg:
        for nm, shp in [("hatm", [128, CD]), ("hatq", [QL, CD]), ("tqA0", [128, CQ]),
                        ("p1", [128, CQ]), ("a1", [128, CQ]), ("p2", [128, CQ]),
                        ("a2", [128, CQ]), ("p3", [128, CQ]), ("mTc1d", [128, C * 128]),
                        ("mTc2d", [25, C * 128]), ("tqB0", [25, CQ])]:
            dbg_d[nm] = nc.dram_tensor("dbg_" + nm, shp, F32, kind="ExternalOutput")

    with tile.TileContext(nc) as tc:
        with (
            nc.allow_low_precision("routing coefficients tolerate bf16"),
            tc.tile_pool(name="sb", bufs=1) as sb,
            tc.tile_pool(name="sb2", bufs=3) as sb2,
        ):
            # ---------------- loads ----------------
            # SP queue: mT then W per k-chunk (the projection stream);
            # Pool queue: qT, eye (needed later) so they don't delay W chunks.
            W_sb = sb.tile([128, KC, NPAD], BF, tag="W")
            mT_sb = sb.tile([128, KC, I], BF, tag="mT")
            qT_sb = sb.tile([128, KC, QL], BF, tag="qT")
            eye = sb.tile([128, 128], F32, tag="eye")
            nc.scalar.dma_start(mT_sb[:], mT_d[:].rearrange("p (k n) -> p k n", k=KC))
            Wr = W_d[:].rearrange("p (k n) -> p k n", k=KC)
            for k in range(KC):
                nc.sync.dma_start(W_sb[:, k, :], Wr[:, k, :])
            nc.scalar.dma_start(qT_sb[:], qT_d[:].rearrange("p (k n) -> p k n", k=KC))
            nc.scalar.dma_start(eye[:], eye_d[:])
            if with_bias:
                b_sb = sb.tile([1, CD], F32, tag="b")
                nc.sync.dma_start(b_sb[:], b_d[:])
            ones1 = sb.tile([1, 128], F32, tag="ones1")
            nc.vector.memset(ones1[:], 1.0)
            ones1B = sb.tile([1, 128], BF, tag="ones1B")
            nc.vector.memset(ones1B[:], 1.0)
            onesD = sb.tile([128, 1], BF, tag="onesD")
            nc.vector.memset(onesD[:], 1.0)
            epsb = sb.tile([128, 1], F32, tag="epsb")
            nc.vector.memset(epsb[:], EPS)

            # ---------------- projections (hat-major) ----------------
            hat_m_r = sb.tile([128, CD + 1], DT, tag="hatmr")  # col 765 zero (even-N pad)
            hat_q32 = sb.tile([QL, CD], F32, tag="hatq32")

            with tc.tile_pool(name="ps1", bufs=1, space="PSUM") as ps1, \
                 tc.tile_pool(name="pstp", bufs=4, space="PSUM") as pstp:
                psA = ps1.tile([128, 512], F32, tag="psA")
                psB = ps1.tile([128, 256], F32, tag="psB")
                for k in range(KC):
                    nc.tensor.matmul(psA[:], mT_sb[:, k, :], W_sb[:, k, 0:512],
                                     start=(k == 0), stop=(k == KC - 1 and not with_bias))
                    nc.tensor.matmul(psB[:], mT_sb[:, k, :], W_sb[:, k, 512:768],
                                     start=(k == 0), stop=(k == KC - 1 and not with_bias))
                if with_bias:
                    nc.tensor.matmul(psA[:], ones1[:], b_sb[:, 0:512], start=False, stop=True)
                    nc.tensor.matmul(psB[:, 0:253], ones1[:], b_sb[:, 512:765],
                                     start=False, stop=True)
                nc.scalar.copy(hat_m_r[:, 0:512], psA[:])
                nc.vector.tensor_copy(hat_m_r[:, 512:765], psB[:, 0:253])
                nc.vector.memset(hat_m_r[:, 765:766].bitcast(F32), 0.0)

                psC = ps1.tile([QL, 512], F32, tag="psC")
                psD = ps1.tile([QL, 256], F32, tag="psD")
                for k in range(KC):
                    nc.tensor.matmul(psC[:], qT_sb[:, k, :], W_sb[:, k, 0:512],
                                     start=(k == 0), stop=(k == KC - 1 and not with_bias))
                    nc.tensor.matmul(psD[:], qT_sb[:, k, :], W_sb[:, k, 512:768],
                                     start=(k == 0), stop=(k == KC - 1 and not with_bias))
                if with_bias:
                    onesq = sb.tile([1, QL], F32, tag="onesq")
                    nc.vector.memset(onesq[:], 1.0)
                    nc.tensor.matmul(psC[:], onesq[:], b_sb[:, 0:512],
                                     start=False, stop=True)
                    nc.tensor.matmul(psD[:, 0:253], onesq[:], b_sb[:, 512:765],
                                     start=False, stop=True)
                # NOTE: bias-for-q path writes b broadcast over q? must be b per column:
                # out[q, n] += 1*b[n] -> lhsT = onesq [1, QL], rhs = b [1, n] OK.
                nc.scalar.copy(hat_q32[:, 0:512], psC[:])
                nc.vector.tensor_copy(hat_q32[:, 512:765], psD[:, 0:253])

                # ---------------- m stats ----------------
                # mum [128, C] on DVE (needed early for centering); xn2 via Act
                # square+accum per c (keeps DVE free for transpose copies)
                hm32 = hat_m_r[:, 0:765].bitcast(F32)
                mum = sb.tile([128, C], F32, tag="mum")
                nc.vector.tensor_reduce(mum[:], hm32.rearrange("p (c d) -> p c d", c=C),
                                        axis=AX, op=ADD)  # holds D*mean
                xn2 = sb.tile([128, C], F32, tag="xn2")
                sqm = sb.tile([128, CD], F32, tag="sqm")
                nc.gpsimd.tensor_tensor(sqm[:], hm32, hm32, op=MUL)
                # bf16 hat_m: moving operand for hv/final matmuls (pairs with
                # bf16 dsp); built on the otherwise-idle Pool engine
                hat_m_bf = sb.tile([128, CD + 1], BF, tag="hatmbf")
                nc.gpsimd.tensor_scalar(hat_m_bf[:], hat_m_r[:].bitcast(F32), 1.0, None, op0=MUL)
                nc.vector.tensor_reduce(xn2[:], sqm[:].rearrange("p (c d) -> p c d", c=C),
                                        axis=AX, op=ADD)
                # xn2 = sum(hm^2) - D*mum^2 ; inv_xn = 1/sqrt(xn2)
                mum2 = sb.tile([128, C], F32, tag="mum2")
                nc.vector.tensor_tensor(mum2[:], mum[:], mum[:], op=MUL)
                nc.vector.tensor_scalar(mum2[:], mum2[:], 1.0 / D, None, op0=MUL)
                nc.vector.tensor_tensor(xn2[:], xn2[:], mum2[:], op=SUB)
                lxn = sb.tile([128, C], F32, tag="lxn")
                nc.scalar.activation(lxn[:], xn2[:], mybir.ActivationFunctionType.Ln)
                inv_xn = sb.tile([128, C], F32, tag="invxn")
                nc.scalar.activation(inv_xn[:], lxn[:], mybir.ActivationFunctionType.Exp, scale=-0.5)

                # mumd = mum/D [128, C]: centering correction applied as
                # num_centered = num_raw - bcast(S1) * mumd (free-dim broadcast)
                mumd = sb.tile([128, C], F32, tag="mumd")
                nc.vector.tensor_scalar(mumd[:], mum[:], 1.0 / D, None, op0=MUL)

                # ---------------- transposes: hmT and tq ----------------
                mTu1 = sb.tile([128, C, 128], BF, tag="mTu1")   # rows d=0..127
                mTu2 = sb.tile([25, C, 128], BF, tag="mTu2")    # rows d=128..152
                tqA = sb.tile([128, C, QL], BF, tag="tqA")
                tqB = sb.tile([25, C, QL], BF, tag="tqB")
                vA = sb.tile([128, C, QL], BF, tag="vA")
                vB = sb.tile([25, C, QL], BF, tag="vB")

                for c in range(C):
                    t1 = pstp.tile([128, 128], F32, tag="tp")
                    nc.tensor.transpose(t1[:], hat_m_r[:, D * c:D * c + 128].bitcast(F32), eye[:])
                    nc.scalar.copy(mTu1[:, c, :], t1[:])
                    t2 = pstp.tile([25, 128], F32, tag="tp")
                    nc.tensor.transpose(t2[:], hat_m_r[:, D * c + 128:D * (c + 1)].bitcast(F32), eye[:])
                    nc.vector.tensor_copy(mTu2[:, c, :], t2[:])

                    t3 = pstp.tile([128, QL], F32, tag="tp")
                    nc.tensor.transpose(t3[:], hat_q32[:, D * c:D * c + 128], eye[0:QL, 0:QL])
                    nc.scalar.copy(tqA[:, c, :], t3[:])
                    t4 = pstp.tile([25, QL], F32, tag="tp")
                    nc.tensor.transpose(t4[:], hat_q32[:, D * c + 128:D * (c + 1)], eye[0:QL, 0:QL])
                    nc.vector.tensor_copy(tqB[:, c, :], t4[:])

            if dbg:
                nc.sync.dma_start(dbg_d["hatm"][:], hat_m_r[:, 0:765].bitcast(F32))
                nc.sync.dma_start(dbg_d["hatq"][:], hat_q32[:])
                nc.sync.dma_start(dbg_d["tqA0"][:], tqA[:].bitcast(F32).rearrange("p c q -> p (c q)"))
                nc.sync.dma_start(dbg_d["mTc1d"][:], mTc1[:].bitcast(F32).rearrange("p c q -> p (c q)"))
                nc.sync.dma_start(dbg_d["mTc2d"][:], mTc2[:].bitcast(F32).rearrange("p c q -> p (c q)"))
            # ---------------- routing ----------------
            with tc.tile_pool(name="ps2", bufs=1, space="PSUM") as ps2:
                LN = mybir.ActivationFunctionType.Ln
                EXP = mybir.ActivationFunctionType.Exp
                CPY = mybir.ActivationFunctionType.Copy
                ixn_b = inv_xn[:].rearrange("p (c a) -> p c a", a=1).broadcast_to((128, C, QL))
                tqAf = tqA[:].rearrange("p c q -> p (c q)")
                tqBf = tqB[:].rearrange("p c q -> p (c q)")
                neg1 = sb.tile([1, 1], BF, tag="neg1")
                nc.vector.memset(neg1[:], -1.0)
                # stage mumd as a [1, C*128] row: the centering correction
                # -mumd x colsum is fused into the num/numh psum groups
                tpm = ps2.tile([C, 128], F32, tag="rows")
                nc.tensor.transpose(tpm[:], mumd[:], eye[:])
                mumdT = sb.tile([C, 128], F32, tag="mumdT")
                nc.scalar.copy(mumdT[:], tpm[:])
                mumrow = sb.tile([1, C, 128], F32, tag="mumrow")
                nc.sync.dma_start(mumrow[:], mumdT[:, :])

                # persistent cross-iteration state (f32, incrementally updated)
                nxS = sb.tile([128, C, QL], F32, tag="nxS")   # (num - S1*mumd)*inv_xn
                yn2S = sb.tile([1, CQ], F32, tag="yn2S")      # sum tq^2 - S1^2/D
                S1S = sb.tile([1, CQ], F32, tag="S1S")        # colsum(tq)

                def p_tail(yn2_src):
                    """p = tanh(nxS * rsqrt(yn2_src))"""
                    lyn = sb2.tile([1, CQ], F32, tag="lyn")
                    nc.scalar.activation(lyn[:], yn2_src, LN)
                    inv_yn = sb2.tile([1, CQ], BF, tag="invyn")
                    nc.scalar.activation(inv_yn[:], lyn[:], EXP, scale=-0.5)
                    iyb = ps2.tile([128, CQ], F32, tag="bcast")
                    nc.tensor.matmul(iyb[:], ones1B[:], inv_yn[:], start=True, stop=True)
                    pp = sb2.tile([128, CQ], F32, tag="pp")
                    nc.vector.tensor_tensor(pp[:], nxS[:].rearrange("p c q -> p (c q)"),
                                            iyb[:], op=MUL)
                    # tanh(x) = 1 - 2/(1+exp(2x))
                    e2 = sb2.tile([128, CQ], F32, tag="e2")
                    nc.scalar.activation(e2[:], pp[:], EXP, scale=2.0)
                    den = sb2.tile([128, CQ], F32, tag="dent")
                    nc.vector.tensor_scalar(den[:], e2[:], 1.0, None, op0=ADD)
                    rr = sb2.tile([128, CQ], F32, tag="rr")
                    nc.vector.reciprocal(rr[:], den[:])
                    p_new = sb2.tile([128, CQ], BF, tag="p")
                    nc.vector.tensor_scalar(p_new[:], rr[:], -2.0, 1.0, op0=MUL, op1=ADD)
                    return p_new

                # ---- pearson #1 (full; initializes S1S / yn2S / nxS) ----
                rows1 = ps2.tile([1, 3 * CQ], F32, tag="rows")
                S1 = rows1[:, 0:CQ]
                S2 = rows1[:, CQ:2 * CQ]
                nc.tensor.matmul(S1, onesD[:, :], tqAf, start=True, stop=False)
                nc.tensor.matmul(S1, onesD[0:25, :], tqBf, start=False, stop=True)
                S1sN = sb2.tile([1, CQ], F32, tag="s1sN")
                nc.scalar.activation(S1sN[:], S1, CPY, scale=-1.0)
                nc.vector.tensor_scalar(S1S[:], S1, 1.0, None, op0=MUL)
                num = ps2.tile([128, C, QL], F32, tag="num")
                for c in range(C):
                    nc.tensor.matmul(num[:, c, :], mTu1[:, c, :], tqA[:, c, :],
                                     start=True, stop=False)
                    nc.tensor.matmul(num[:, c, :], mTu2[:, c, :], tqB[:, c, :],
                                     start=False, stop=False)
                    nc.tensor.matmul(num[:, c, :], mumrow[:, c, :],
                                     S1sN[:, QL * c:QL * (c + 1)],
                                     start=False, stop=True)
                sqA = sb2.tile([128, CQ], BF, tag="sqA")
                nc.vector.tensor_tensor(sqA[:], tqAf, tqAf, op=MUL)
                sqB = sb2.tile([25, CQ], BF, tag="sqB")
                nc.vector.tensor_tensor(sqB[:], tqBf, tqBf, op=MUL)
                sq1 = sb2.tile([1, CQ], BF, tag="sq1")
                nc.scalar.activation(sq1[:], S1, mybir.ActivationFunctionType.Square,
                                     scale=D ** -0.5)
                nc.tensor.matmul(S2, onesD[:, :], sqA[:], start=True, stop=False)
                nc.tensor.matmul(S2, onesD[0:25, :], sqB[:], start=False, stop=False)
                nc.tensor.matmul(S2, neg1[:], sq1[:], start=False, stop=True)
                nc.vector.tensor_tensor(nxS[:], num[:], ixn_b, op=MUL)
                nc.vector.tensor_scalar(yn2S[:], S2, 1.0, None, op0=MUL)
                p_t = p_tail(S2)
                if dbg:
                    nc.sync.dma_start(dbg_d["p1"][:], p_t[:])
                a_t = None

                for it in range(2):
                    gf = float(1 << it)      # tq is unnormalized: update adds (2^it s) hv
                    dsp = sb2.tile([128, C, QL], BF, tag="dsp")
                    if it == 0:
                        # softmax(0) = 1/C exactly
                        nc.vector.tensor_scalar(dsp[:].rearrange("p c q -> p (c q)"),
                                                p_t[:], 1.0 / C, None, op0=ADD)
                    else:
                        ea = sb2.tile([128, CQ], BF, tag="ea")
                        nc.scalar.activation(ea[:], a_t[:], EXP)
                        asum = sb2.tile([128, QL], BF, tag="asum")
                        nc.vector.tensor_reduce(asum[:], ea[:].rearrange("p (c q) -> p q c", c=C),
                                                axis=AX, op=ADD)
                        rs = sb2.tile([128, QL], BF, tag="rs")
                        nc.vector.reciprocal(rs[:], asum[:])
                        dd = sb2.tile([128, C, QL], BF, tag="dd")
                        nc.vector.tensor_tensor(
                            dd[:], ea[:].rearrange("p (c q) -> p c q", c=C),
                            rs[:].rearrange("p (a q) -> p a q", a=1).broadcast_to((128, C, QL)),
                            op=MUL)
                        nc.vector.tensor_tensor(dsp[:].rearrange("p c q -> p (c q)"),
                                                dd[:].rearrange("p c q -> p (c q)"), p_t[:], op=ADD)

                    # hv[d, (c,q)] in two D-chunks
                    hvA = ps2.tile([128, C, QL], F32, tag="hvA")
                    hvB = ps2.tile([26, C, QL], F32, tag="hvB")
                    for c in range(C):
                        nc.tensor.matmul(hvA[:, c, :], hat_m_bf[:, D * c:D * c + 128], dsp[:, c, :],
                                         start=True, stop=True)
                        nc.tensor.matmul(hvB[:, c, :], hat_m_bf[:, D * c + 128:D * c + 154], dsp[:, c, :],
                                         start=True, stop=True)
                    vAf = vA[:].rearrange("p c q -> p (c q)")
                    vBf = vB[:].rearrange("p c q -> p (c q)")
                    hvAf = hvA[:].rearrange("p c q -> p (c q)")
                    hvBf = hvB[0:25].rearrange("p c q -> p (c q)")
                    nc.scalar.copy(vA[:].rearrange("p c q -> p (c q)"), hvAf)
                    nc.vector.tensor_copy(vB[:].rearrange("p c q -> p (c q)"), hvBf)
                    sqhA = sb2.tile([128, CQ], BF, tag="sqhA")
                    nc.vector.tensor_tensor(sqhA[:], vAf, vAf, op=MUL)
                    sqhB = sb2.tile([25, CQ], BF, tag="sqhB")
                    nc.vector.tensor_tensor(sqhB[:], vBf, vBf, op=MUL)
                    rowsI = ps2.tile([1, 3 * CQ], F32, tag="rows")
                    n2 = rowsI[:, 0:CQ]
                    H1 = rowsI[:, CQ:2 * CQ]
                    X = rowsI[:, 2 * CQ:3 * CQ]
                    nc.tensor.matmul(n2, onesD[:, :], sqhA[:], start=True, stop=False)
                    nc.tensor.matmul(n2, onesD[0:25, :], sqhB[:], start=False, stop=True)
                    nc.tensor.matmul(H1, onesD[:, :], vAf, start=True, stop=False)
                    nc.tensor.matmul(H1, onesD[0:25, :], vBf, start=False, stop=True)
                    tqhA = sb2.tile([128, CQ], BF, tag="tqhA")
                    nc.vector.tensor_tensor(tqhA[:], tqAf, vAf, op=MUL)
                    tqhB = sb2.tile([25, CQ], BF, tag="tqhB")
                    nc.vector.tensor_tensor(tqhB[:], tqBf, vBf, op=MUL)
                    nc.tensor.matmul(X, onesD[:, :], tqhA[:], start=True, stop=False)
                    nc.tensor.matmul(X, onesD[0:25, :], tqhB[:], start=False, stop=True)
                    # H1sN = -H1 staged once (serves centering + row terms)
                    H1sN = sb2.tile([1, CQ], F32, tag="H1sN")
                    nc.scalar.activation(H1sN[:], H1, CPY, scale=-1.0)
                    # early row terms (pre-srow):
                    #   Ag = 2 gf (X - S1S*H1/D),  Bg = gf^2 (n2 - H1^2/D)
                    c1 = sb2.tile([1, CQ], F32, tag="c1")
                    nc.vector.tensor_tensor(c1[:], S1S[:], H1sN[:], op=MUL)   # = -S1*H1
                    c1s = sb2.tile([1, CQ], F32, tag="c1s")
                    nc.vector.tensor_scalar(c1s[:], c1[:], 2.0 * gf / D, None, op0=MUL)
                    X2 = sb2.tile([1, CQ], F32, tag="X2")
                    nc.vector.tensor_scalar(X2[:], X, 2.0 * gf, None, op0=MUL)
                    Ag = sb2.tile([1, CQ], F32, tag="Ag")
                    nc.vector.tensor_tensor(Ag[:], X2[:], c1s[:], op=ADD)
                    sqH = sb2.tile([1, CQ], F32, tag="sqH")
                    nc.vector.tensor_tensor(sqH[:], H1sN[:], H1sN[:], op=MUL)
                    sqHs = sb2.tile([1, CQ], F32, tag="sqHs")
                    nc.vector.tensor_scalar(sqHs[:], sqH[:], gf * gf / D, None, op0=MUL)
                    n2g = sb2.tile([1, CQ], F32, tag="n2g")
                    nc.vector.tensor_scalar(n2g[:], n2, gf * gf, None, op0=MUL)
                    Bg = sb2.tile([1, CQ], F32, tag="Bg")
                    nc.vector.tensor_tensor(Bg[:], n2g[:], sqHs[:], op=SUB)
                    # numh (centered via fused -mumd x H1) -> nxh = gf*numh*inv_xn
                    numh = ps2.tile([128, C, QL], F32, tag="num")
                    for c in range(C):
                        nc.tensor.matmul(numh[:, c, :], mTu1[:, c, :], vA[:, c, :],
                                         start=True, stop=False)
                        nc.tensor.matmul(numh[:, c, :], mTu2[:, c, :], vB[:, c, :],
                                         start=False, stop=False)
                        nc.tensor.matmul(numh[:, c, :], mumrow[:, c, :],
                                         H1sN[:, QL * c:QL * (c + 1)],
                                         start=False, stop=True)
                    nxh = sb2.tile([128, C, QL], F32, tag="nxh")
                    nc.vector.tensor_tensor(nxh[:], numh[:], ixn_b, op=MUL)
                    if it == 1:
                        nc.vector.tensor_scalar(nxh[:].rearrange("p c q -> p (c q)"),
                                                nxh[:].rearrange("p c q -> p (c q)"),
                                                gf, None, op0=MUL)
                    # mdv (uncentered m); pm = p*mdv early
                    mdv = ps2.tile([128, C, QL], F32, tag="mdv")
                    for c in range(C):
                        nc.tensor.matmul(mdv[:, c, :], mTu1[:, c, :], vA[:, c, :],
                                         start=True, stop=False)
                        nc.tensor.matmul(mdv[:, c, :], mTu2[:, c, :], vB[:, c, :],
                                         start=False, stop=True)
                    pm = sb2.tile([128, CQ], F32, tag="pm")
                    nc.vector.tensor_tensor(pm[:], mdv[:].rearrange("p c q -> p (c q)"), p_t[:], op=MUL)

                    # squash scale row: s = sqrt(n2)/(1+n2)
                    n2p1 = sb2.tile([1, CQ], F32, tag="n2p1")
                    nc.vector.tensor_scalar(n2p1[:], n2, 1.0, None, op0=ADD)
                    r1 = sb2.tile([1, CQ], F32, tag="r1")
                    nc.vector.reciprocal(r1[:], n2p1[:])
                    ln2 = sb2.tile([1, CQ], F32, tag="ln2")
                    nc.scalar.activation(ln2[:], n2, LN, bias=epsb[0:1, :])
                    sqn = sb2.tile([1, CQ], BF, tag="sqn")
                    nc.scalar.activation(sqn[:], ln2[:], EXP, scale=0.5)
                    srow = sb2.tile([1, CQ], BF, tag="srow")
                    nc.vector.tensor_tensor(srow[:], sqn[:], r1[:], op=MUL)
                    sB = ps2.tile([128, CQ], F32, tag="bcast")
                    nc.tensor.matmul(sB[:], ones1B[:], srow[:], start=True, stop=True)

                    # yn2S += s*Ag + s^2*Bg  (-> next pearson's denominator)
                    u1 = sb2.tile([1, CQ], F32, tag="u1")
                    nc.vector.tensor_tensor(u1[:], srow[:], Ag[:], op=MUL)
                    sq_s = sb2.tile([1, CQ], F32, tag="sq_s")
                    nc.vector.tensor_tensor(sq_s[:], srow[:], srow[:], op=MUL)
                    u2 = sb2.tile([1, CQ], F32, tag="u2")
                    nc.vector.tensor_tensor(u2[:], sq_s[:], Bg[:], op=MUL)
                    w = sb2.tile([1, CQ], F32, tag="w")
                    nc.vector.tensor_tensor(w[:], u1[:], u2[:], op=ADD)
                    nc.vector.tensor_tensor(yn2S[:], yn2S[:], w[:], op=ADD)
                    # S1S += gf * s * H1  (slack: needed next iteration only)
                    H1g = sb2.tile([1, CQ], F32, tag="H1g")
                    nc.vector.tensor_scalar(H1g[:], H1sN[:], -gf, None, op0=MUL)
                    sh = sb2.tile([1, CQ], F32, tag="sh")
                    nc.vector.tensor_tensor(sh[:], srow[:], H1g[:], op=MUL)
                    nc.vector.tensor_tensor(S1S[:], S1S[:], sh[:], op=ADD)
                    # nxS += s * nxh
                    nupd = sb2.tile([128, CQ], F32, tag="nupd")
                    nc.vector.tensor_tensor(nupd[:], sB[:], nxh[:].rearrange("p c q -> p (c q)"), op=MUL)
                    nc.vector.tensor_tensor(nxS[:].rearrange("p c q -> p (c q)"),
                                            nxS[:].rearrange("p c q -> p (c q)"), nupd[:], op=ADD)
                    # a += p * s * mdv
                    pms = sb2.tile([128, CQ], BF, tag="pms")
                    nc.vector.tensor_tensor(pms[:], pm[:], sB[:], op=MUL)
                    if it == 0:
                        a_t = pms
                    else:
                        a_new = sb2.tile([128, CQ], BF, tag="a")
                        nc.vector.tensor_tensor(a_new[:], a_t[:], pms[:], op=ADD)
                        a_t = a_new
                    # tq += (gf * s) * hv  (slack: feeds next iteration's X/S1 terms)
                    svA = sb2.tile([128, CQ], BF, tag="svA")
                    nc.vector.tensor_tensor(svA[:], vAf, sB[:], op=MUL)
                    svB = sb2.tile([25, CQ], BF, tag="svB")
                    nc.vector.tensor_tensor(svB[:], vBf, sB[0:25, :], op=MUL)
                    if it == 1:
                        nc.vector.tensor_scalar(svA[:], svA[:], 2.0, None, op0=MUL)
                        nc.vector.tensor_scalar(svB[:], svB[:], 2.0, None, op0=MUL)
                    nc.vector.tensor_tensor(tqAf, tqAf, svA[:], op=ADD)
                    nc.vector.tensor_tensor(tqBf, tqBf, svB[:], op=ADD)

                    p_t = p_tail(yn2S[:])
                    if dbg:
                        nc.sync.dma_start(dbg_d["a1" if it == 0 else "a2"][:], a_t[:])
                        nc.sync.dma_start(dbg_d["p2" if it == 0 else "p3"][:], p_t[:])

                # ---------------- final softmax ----------------
                ea = sb2.tile([128, CQ], BF, tag="ea")
                nc.scalar.activation(ea[:], a_t[:], EXP)
                asum = sb2.tile([128, QL], BF, tag="asum")
                nc.vector.tensor_reduce(asum[:], ea[:].rearrange("p (c q) -> p q c", c=C),
                                        axis=AX, op=ADD)
                rs = sb2.tile([128, QL], BF, tag="rs")
                nc.vector.reciprocal(rs[:], asum[:])
                dd = sb2.tile([128, C, QL], BF, tag="dd")
                nc.vector.tensor_tensor(
                    dd[:], ea[:].rearrange("p (c q) -> p c q", c=C),
                    rs[:].rearrange("p (a q) -> p a q", a=1).broadcast_to((128, C, QL)), op=MUL)
                dspF = sb2.tile([128, C, QL], BF, tag="dspbf")
                nc.vector.tensor_tensor(dspF[:].rearrange("p c q -> p (c q)"),
                                        dd[:].rearrange("p c q -> p (c q)"), p_t[:], op=ADD)

                # ---------------- final ----------------
                # per-c: matmul -> copy + square (Pool) + reduce (DVE), pipelined
                hvF = sb.tile([QL, CD], F32, tag="hvF")
                n2q = sb2.tile([QL, C], F32, tag="n2q")
                fsq = sb2.tile([QL, D], F32, tag="fsq")
                for c in range(C):
                    fps = ps2.tile([QL, D + 1], F32, tag=("hvA" if c % 2 == 0 else "mdv"))
                    nc.tensor.matmul(fps[:], dspF[:, c, :], hat_m_bf[:, D * c:D * c + 154],
                                     start=True, stop=True)
                    nc.vector.tensor_copy(hvF[:, D * c:D * (c + 1)], fps[:, 0:153])
                    nc.scalar.activation(fsq[:], fps[:, 0:153],
                                         mybir.ActivationFunctionType.Square,
                                         accum_out=n2q[:, c:c + 1])
                fp1 = sb2.tile([QL, C], F32, tag="fp1")
                nc.vector.tensor_scalar(fp1[:], n2q[:], 1.0, None, op0=ADD)
                fr1 = sb2.tile([QL, C], F32, tag="fr1")
                nc.vector.reciprocal(fr1[:], fp1[:])
                fln = sb2.tile([QL, C], F32, tag="fln")
                nc.scalar.activation(fln[:], n2q[:], mybir.ActivationFunctionType.Ln, bias=epsb[0:QL, :])
                fr2 = sb2.tile([QL, C], F32, tag="fr2")
                nc.scalar.activation(fr2[:], fln[:], mybir.ActivationFunctionType.Exp, scale=-0.5)
                fs1 = sb2.tile([QL, C], F32, tag="fs1")
                nc.vector.tensor_scalar(fs1[:], fr1[:], -1.0, 1.0, op0=MUL, op1=ADD)
                fs = sb2.tile([QL, C], F32, tag="fs")
                nc.vector.tensor_tensor(fs[:], fs1[:], fr2[:], op=MUL)
                # out = hvF * fs: c=0..2 on DVE -> sync DMA; c=3,4 on Act ->
                # scalar-queue DMA (same queue as producer: no cross sem)
                outT = sb.tile([QL, CD], F32, tag="outT")
                D3 = 3 * D
                nc.vector.tensor_tensor(
                    outT[:, 0:D3].rearrange("p (c d) -> p c d", c=3),
                    hvF[:, 0:D3].rearrange("p (c d) -> p c d", c=3),
                    fs[:, 0:3].rearrange("p (c a) -> p c a", a=1).broadcast_to((QL, 3, D)), op=MUL)
                nc.sync.dma_start(out_d[:, 0:D3], outT[:, 0:D3])
                for c in (3, 4):
                    nc.scalar.activation(outT[:, D * c:D * (c + 1)], hvF[:, D * c:D * (c + 1)],
                                         mybir.ActivationFunctionType.Copy, scale=fs[:, c:c + 1])
                nc.scalar.dma_start(out_d[:, D3:CD], outT[:, D3:CD])

    # All activations use only {Ln, Exp, Copy}, which live together in act
    # func set 6 (natural_log_exp_and_others). The default solver alternates
    # sets 0/5, inserting ~15 table reloads (~1.3us each); one load suffices.
    def _single_act_table_load():
        inst = mybir.InstLoadActFuncSet(
            name=nc.get_next_instruction_name(), ins=[], outs=[],
            act_func_set_id=6,
        )
        inst.engine = mybir.EngineType.Activation
        nc.register_instruction(inst)
        for blk in nc.main_func.blocks:
            for idx, bi in enumerate(blk.instructions):
                if isinstance(bi, mybir.InstActivation):
                    blk.instructions.insert(idx, inst)
                    return
        raise AssertionError("no activation found")

    nc.insert_act_table_loads = _single_act_table_load
    nc.compile()
    return nc


_CACHE = {}
LAST_EXEC_NS = None
LAST_RESULTS = None


def kernel(m, q, W, b):
    m = np.asarray(m, dtype=np.float32)
    q = np.asarray(q, dtype=np.float32)
    W = np.asarray(W, dtype=np.float32)
    b = np.asarray(b, dtype=np.float32)
    assert m.shape == (I, K) and q.shape == (NCORES * QL, K) and W.shape == (K, CD)

    with_bias = bool(np.any(b))
    dbg = bool(int(os.environ.get("KERNEL_DBG", "0")))
    key = ("v1", with_bias, str(DT), dbg)
    if key not in _CACHE:
        _CACHE[key] = build(with_bias, dbg)
    nc = _CACHE[key]

    import ml_dtypes
    BF = ml_dtypes.bfloat16

    def pre(x, n):  # [K, n] -> p-major [128, KC*n] bf16
        return np.ascontiguousarray(
            x.reshape(KC, 128, n).transpose(1, 0, 2).reshape(128, KC * n)).astype(BF)

    Wp = np.zeros((K, NPAD), dtype=np.float32)
    Wp[:, :CD] = W
    W_pre = pre(Wp, NPAD)
    mT_pre = pre(np.ascontiguousarray(m.T), I)
    eye = np.eye(128, dtype=np.float32)
    b2 = b.reshape(1, CD)

    in_maps = []
    for i in range(NCORES):
        qT_pre = pre(np.ascontiguousarray(q[QL * i:QL * (i + 1)].T), QL)
        in_maps.append({"mTp": mT_pre, "qTp": qT_pre, "Wpp": W_pre, "b": b2, "eye": eye})

    res = run_bass_kernel_spmd(nc, in_maps, list(range(NCORES)))
    global LAST_EXEC_NS, LAST_RESULTS
    LAST_EXEC_NS = res.exec_time_ns
    LAST_RESULTS = res.results
    out = np.concatenate([res.results[i]["out"] for i in range(NCORES)], axis=0)
    return out.astype(np.float32)


if __name__ == "__main__":
    rng = np.random.default_rng(0)
    m = rng.standard_normal((I, K)).astype(np.float32)
    q = rng.standard_normal((NCORES * QL, K)).astype(np.float32)
    W = (rng.standard_normal((K, CD)) * 0.02).astype(np.float32)
    b = np.zeros((CD,), dtype=np.float32)
    out = kernel(m=m, q=q, W=W, b=b)
    print("out", out.shape, out.dtype, np.abs(out).mean())



# revision 46
# speedup vs baseline: 1.0264x; 1.0264x over previous
"""DMR induction routing kernel for Trainium2 (Bass/Tile), 8-core data-parallel.

Problem: nn_DMRInduction. Full inputs:
  m [128, 768], q [256, 768], W [768, 765], b [765] -> out [256, 765] fp32.

Sharding: Q=256 split 8 ways (32 queries/core); m, W, b replicated.

Per-core layouts:
  - hat_m        [I=128, C*D=765]   (I on partitions)  - hv weights / final hv rhs
  - hmT aug      [D+1=154, I] per c (D on partitions)  - num/mdv weights;
      row 153 holds -mean_c(m) so the num matmul computes the centered
      correlation numerator directly (sum_d xm*tq = sum_d m*tq - mum*colsum).
  - tq, v        [D, C*Q=160] as two tiles [128,160] + [34,160]
      (tqB row 32 carries colsum for the augmented num matmul and the
       yn2 correction; vB rows 25..33 stay zero so mdv stays uncentered).
  - routing state a, p, dsp [I=128, C*Q=160].
  - final hat_v  [Q=32, C*D=765] -> squash -> contiguous DMA out.
"""
import os
import sys

for _p in ("/opt/trn_rl_repo", "/root/.axon_site/_ro/trn_rl_repo"):
    if os.path.isdir(_p) and _p not in sys.path:
        sys.path.insert(0, _p)

import numpy as np
import concourse.bass as bass
import concourse.bacc as bacc
import concourse.mybir as mybir
import concourse.tile as tile
from concourse.bass_utils import run_bass_kernel_spmd

F32 = mybir.dt.float32
# Matmul input dtype. float32 is exact (final scale-relative err ~2e-5);
# float32r uses the fast PE path (1 cyc/row at N>=256 vs 4) and cuts the
# projection phase ~14us, at ~2.5e-4 scale-relative output error. The
# rest of the kernel is dependency-latency-bound, so the dtype only
# affects the projection matmuls. Default to exact.
DT = getattr(mybir.dt, os.environ.get("KERNEL_MM_DT", "float32r"))

NCORES = 8
I = 128         # memory capsules
C = 5           # capsule classes
D = 153         # dim per capsule
CD = C * D      # 765
K = 768         # input dim
KC = K // 128   # 6 contraction chunks
QL = 32         # queries per core
CQ = C * QL     # 160
NPAD = 768      # W padded to 768 cols so fp32r matmuls stream N>=256
EPS = 1e-8
AX = mybir.AxisListType.X
MUL = mybir.AluOpType.mult
ADD = mybir.AluOpType.add
SUB = mybir.AluOpType.subtract


def build(with_bias: bool, dbg: bool = False):
    nc = bacc.Bacc("TRN2", target_bir_lowering=False, debug=False)

    BF = mybir.dt.bfloat16
    # inputs arrive host-prearranged p-major so each load is one contiguous
    # descriptor per partition: x_pre[p, k, n] = x[k*128+p, n]
    mT_d = nc.dram_tensor("mTp", [128, KC * I], BF, kind="ExternalInput")
    qT_d = nc.dram_tensor("qTp", [128, KC * QL], BF, kind="ExternalInput")
    W_d = nc.dram_tensor("Wpp", [128, KC * NPAD], BF, kind="ExternalInput")
    b_d = nc.dram_tensor("b", [1, CD], F32, kind="ExternalInput")
    eye_d = nc.dram_tensor("eye", [128, 128], F32, kind="ExternalInput")
    out_d = nc.dram_tensor("out", [QL, CD], F32, kind="ExternalOutput")
    dbg_d = {}
    if dbg:
        for nm, shp in [("hatm", [128, CD]), ("hatq", [QL, CD]), ("tqA0", [128, CQ]),
                        ("p1", [128, CQ]), ("a1", [128, CQ]), ("p2", [128, CQ]),
                        ("a2", [128, CQ]), ("p3", [128, CQ]), ("mTc1d", [128, C * 128]),
                        ("mTc2d", [25, C * 128]), ("tqB0", [25, CQ])]:
            dbg_d[nm] = nc.dram_tensor("dbg_" + nm, shp, F32, kind="ExternalOutput")

    with tile.TileContext(nc) as tc:
        with (
            nc.allow_low_precision("routing coefficients tolerate bf16"),
            tc.tile_pool(name="sb", bufs=1) as sb,
            tc.tile_pool(name="sb2", bufs=3) as sb2,
        ):
            # ---------------- loads ----------------
            # SP queue: mT then W per k-chunk (the projection stream);
            # Pool queue: qT, eye (needed later) so they don't delay W chunks.
            W_sb = sb.tile([128, KC, NPAD], BF, tag="W")
            mT_sb = sb.tile([128, KC, I], BF, tag="mT")
            qT_sb = sb.tile([128, KC, QL], BF, tag="qT")
            eye = sb.tile([128, 128], F32, tag="eye")
            nc.sync.dma_start(mT_sb[:], mT_d[:].rearrange("p (k n) -> p k n", k=KC))
            Wr = W_d[:].rearrange("p (k n) -> p k n", k=KC)
            for k in range(KC - 1):
                nc.sync.dma_start(W_sb[:, k, :], Wr[:, k, :])
            nc.sync.dma_start(qT_sb[:], qT_d[:].rearrange("p (k n) -> p k n", k=KC))
            nc.sync.dma_start(W_sb[:, KC - 1, :], Wr[:, KC - 1, :])
            nc.sync.dma_start(eye[:], eye_d[:])
            if with_bias:
                b_sb = sb.tile([1, CD], F32, tag="b")
                nc.sync.dma_start(b_sb[:], b_d[:])
            ones1 = sb.tile([1, 128], F32, tag="ones1")
            nc.vector.memset(ones1[:], 1.0)
            ones1B = sb.tile([1, 128], BF, tag="ones1B")
            nc.vector.memset(ones1B[:], 1.0)
            onesD = sb.tile([128, 1], BF, tag="onesD")
            nc.vector.memset(onesD[:], 1.0)
            epsb = sb.tile([128, 1], F32, tag="epsb")
            nc.vector.memset(epsb[:], EPS)

            # ---------------- projections (hat-major) ----------------
            hat_m_r = sb.tile([128, CD + 1], DT, tag="hatmr")  # col 765 zero (even-N pad)
            hat_q32 = sb.tile([QL, CD], F32, tag="hatq32")

            with tc.tile_pool(name="ps1", bufs=1, space="PSUM") as ps1, \
                 tc.tile_pool(name="pstp", bufs=4, space="PSUM") as pstp:
                psA = ps1.tile([128, 512], F32, tag="psA")
                psB = ps1.tile([128, 256], F32, tag="psB")
                for k in range(KC):
                    nc.tensor.matmul(psA[:], mT_sb[:, k, :], W_sb[:, k, 0:512],
                                     start=(k == 0), stop=(k == KC - 1 and not with_bias))
                    nc.tensor.matmul(psB[:], mT_sb[:, k, :], W_sb[:, k, 512:768],
                                     start=(k == 0), stop=(k == KC - 1 and not with_bias))
                if with_bias:
                    nc.tensor.matmul(psA[:], ones1[:], b_sb[:, 0:512], start=False, stop=True)
                    nc.tensor.matmul(psB[:, 0:253], ones1[:], b_sb[:, 512:765],
                                     start=False, stop=True)
                nc.scalar.copy(hat_m_r[:, 0:512], psA[:])
                nc.vector.tensor_copy(hat_m_r[:, 512:765], psB[:, 0:253])
                nc.vector.memset(hat_m_r[:, 765:766].bitcast(F32), 0.0)

                psC = ps1.tile([QL, 512], F32, tag="psC")
                psD = ps1.tile([QL, 256], F32, tag="psD")
                for k in range(KC):
                    nc.tensor.matmul(psC[:], qT_sb[:, k, :], W_sb[:, k, 0:512],
                                     start=(k == 0), stop=(k == KC - 1 and not with_bias))
                    nc.tensor.matmul(psD[:], qT_sb[:, k, :], W_sb[:, k, 512:768],
                                     start=(k == 0), stop=(k == KC - 1 and not with_bias))
                if with_bias:
                    onesq = sb.tile([1, QL], F32, tag="onesq")
                    nc.vector.memset(onesq[:], 1.0)
                    nc.tensor.matmul(psC[:], onesq[:], b_sb[:, 0:512],
                                     start=False, stop=True)
                    nc.tensor.matmul(psD[:, 0:253], onesq[:], b_sb[:, 512:765],
                                     start=False, stop=True)
                # NOTE: bias-for-q path writes b broadcast over q? must be b per column:
                # out[q, n] += 1*b[n] -> lhsT = onesq [1, QL], rhs = b [1, n] OK.
                nc.scalar.copy(hat_q32[:, 0:512], psC[:])
                nc.vector.tensor_copy(hat_q32[:, 512:765], psD[:, 0:253])

                # ---------------- m stats ----------------
                # mum [128, C] on DVE (needed early for centering); xn2 via Act
                # square+accum per c (keeps DVE free for transpose copies)
                hm32 = hat_m_r[:, 0:765].bitcast(F32)
                mum = sb.tile([128, C], F32, tag="mum")
                nc.vector.tensor_reduce(mum[:], hm32.rearrange("p (c d) -> p c d", c=C),
                                        axis=AX, op=ADD)  # holds D*mean
                xn2 = sb.tile([128, C], F32, tag="xn2")
                sqm = sb.tile([128, CD], F32, tag="sqm")
                nc.gpsimd.tensor_tensor(sqm[:], hm32, hm32, op=MUL)
                # bf16 hat_m: moving operand for hv/final matmuls (pairs with
                # bf16 dsp); built on the otherwise-idle Pool engine
                hat_m_bf = sb.tile([128, CD + 1], BF, tag="hatmbf")
                nc.gpsimd.tensor_scalar(hat_m_bf[:], hat_m_r[:].bitcast(F32), 1.0, None, op0=MUL)
                nc.vector.tensor_reduce(xn2[:], sqm[:].rearrange("p (c d) -> p c d", c=C),
                                        axis=AX, op=ADD)
                # xn2 = sum(hm^2) - D*mum^2 ; inv_xn = 1/sqrt(xn2)
                mum2 = sb.tile([128, C], F32, tag="mum2")
                nc.vector.tensor_tensor(mum2[:], mum[:], mum[:], op=MUL)
                nc.vector.tensor_scalar(mum2[:], mum2[:], 1.0 / D, None, op0=MUL)
                nc.vector.tensor_tensor(xn2[:], xn2[:], mum2[:], op=SUB)
                lxn = sb.tile([128, C], F32, tag="lxn")
                nc.scalar.activation(lxn[:], xn2[:], mybir.ActivationFunctionType.Ln)
                inv_xn = sb.tile([128, C], F32, tag="invxn")
                nc.scalar.activation(inv_xn[:], lxn[:], mybir.ActivationFunctionType.Exp, scale=-0.5)

                # mumd = mum/D [128, C]: centering correction applied as
                # num_centered = num_raw - bcast(S1) * mumd (free-dim broadcast)
                mumd = sb.tile([128, C], F32, tag="mumd")
                nc.vector.tensor_scalar(mumd[:], mum[:], 1.0 / D, None, op0=MUL)

                # ---------------- transposes: hmT and tq ----------------
                mTu1 = sb.tile([128, C, 128], BF, tag="mTu1")   # rows d=0..127
                mTu2 = sb.tile([25, C, 128], BF, tag="mTu2")    # rows d=128..152
                tqA = sb.tile([128, C, QL], BF, tag="tqA")
                tqB = sb.tile([25, C, QL], BF, tag="tqB")
                vA = sb.tile([128, C, QL], BF, tag="vA")
                vB = sb.tile([25, C, QL], BF, tag="vB")

                for c in range(C):
                    t1 = pstp.tile([128, 128], F32, tag="tp")
                    nc.tensor.transpose(t1[:], hat_m_r[:, D * c:D * c + 128].bitcast(F32), eye[:])
                    nc.scalar.copy(mTu1[:, c, :], t1[:])
                    t2 = pstp.tile([25, 128], F32, tag="tp")
                    nc.tensor.transpose(t2[:], hat_m_r[:, D * c + 128:D * (c + 1)].bitcast(F32), eye[:])
                    nc.vector.tensor_copy(mTu2[:, c, :], t2[:])

                    t3 = pstp.tile([128, QL], F32, tag="tp")
                    nc.tensor.transpose(t3[:], hat_q32[:, D * c:D * c + 128], eye[0:QL, 0:QL])
                    nc.scalar.copy(tqA[:, c, :], t3[:])
                    t4 = pstp.tile([25, QL], F32, tag="tp")
                    nc.tensor.transpose(t4[:], hat_q32[:, D * c + 128:D * (c + 1)], eye[0:QL, 0:QL])
                    nc.vector.tensor_copy(tqB[:, c, :], t4[:])

            if dbg:
                nc.sync.dma_start(dbg_d["hatm"][:], hat_m_r[:, 0:765].bitcast(F32))
                nc.sync.dma_start(dbg_d["hatq"][:], hat_q32[:])
                nc.sync.dma_start(dbg_d["tqA0"][:], tqA[:].bitcast(F32).rearrange("p c q -> p (c q)"))
                nc.sync.dma_start(dbg_d["mTc1d"][:], mTc1[:].bitcast(F32).rearrange("p c q -> p (c q)"))
                nc.sync.dma_start(dbg_d["mTc2d"][:], mTc2[:].bitcast(F32).rearrange("p c q -> p (c q)"))
            # ---------------- routing ----------------
            with tc.tile_pool(name="ps2", bufs=1, space="PSUM") as ps2:
                LN = mybir.ActivationFunctionType.Ln
                EXP = mybir.ActivationFunctionType.Exp
                CPY = mybir.ActivationFunctionType.Copy
                ixn_b = inv_xn[:].rearrange("p (c a) -> p c a", a=1).broadcast_to((128, C, QL))
                tqAf = tqA[:].rearrange("p c q -> p (c q)")
                tqBf = tqB[:].rearrange("p c q -> p (c q)")
                neg1 = sb.tile([1, 1], BF, tag="neg1")
                nc.vector.memset(neg1[:], -1.0)
                # stage mumd as a [1, C*128] row: the centering correction
                # -mumd x colsum is fused into the num/numh psum groups
                tpm = ps2.tile([C, 128], F32, tag="rows")
                nc.tensor.transpose(tpm[:], mumd[:], eye[:])
                mumdT = sb.tile([C, 128], F32, tag="mumdT")
                nc.scalar.copy(mumdT[:], tpm[:])
                mumrow = sb.tile([1, C, 128], F32, tag="mumrow")
                nc.sync.dma_start(mumrow[:], mumdT[:, :])

                # persistent cross-iteration state (f32, incrementally updated)
                nxS = sb.tile([128, C, QL], F32, tag="nxS")   # (num - S1*mumd)*inv_xn
                yn2S = sb.tile([1, CQ], F32, tag="yn2S")      # sum tq^2 - S1^2/D
                S1S = sb.tile([1, CQ], F32, tag="S1S")        # colsum(tq)

                def p_tail(yn2_src):
                    """p = tanh(nxS * rsqrt(yn2_src))"""
                    lyn = sb2.tile([1, CQ], F32, tag="lyn")
                    nc.scalar.activation(lyn[:], yn2_src, LN)
                    inv_yn = sb2.tile([1, CQ], BF, tag="invyn")
                    nc.scalar.activation(inv_yn[:], lyn[:], EXP, scale=-0.5)
                    iyb = ps2.tile([128, CQ], F32, tag="bcast")
                    nc.tensor.matmul(iyb[:], ones1B[:], inv_yn[:], start=True, stop=True)
                    pp = sb2.tile([128, CQ], F32, tag="pp")
                    nc.vector.tensor_tensor(pp[:], nxS[:].rearrange("p c q -> p (c q)"),
                                            iyb[:], op=MUL)
                    # tanh(x) = 1 - 2/(1+exp(2x))
                    e2 = sb2.tile([128, CQ], F32, tag="e2")
                    nc.scalar.activation(e2[:], pp[:], EXP, scale=2.0)
                    den = sb2.tile([128, CQ], F32, tag="dent")
                    nc.vector.tensor_scalar(den[:], e2[:], 1.0, None, op0=ADD)
                    rr = sb2.tile([128, CQ], F32, tag="rr")
                    nc.vector.reciprocal(rr[:], den[:])
                    p_new = sb2.tile([128, CQ], BF, tag="p")
                    nc.vector.tensor_scalar(p_new[:], rr[:], -2.0, 1.0, op0=MUL, op1=ADD)
                    return p_new

                # ---- pearson #1 (full; initializes S1S / yn2S / nxS) ----
                rows1 = ps2.tile([1, 3 * CQ], F32, tag="rows")
                S1 = rows1[:, 0:CQ]
                S2 = rows1[:, CQ:2 * CQ]
                nc.tensor.matmul(S1, onesD[:, :], tqAf, start=True, stop=False)
                nc.tensor.matmul(S1, onesD[0:25, :], tqBf, start=False, stop=True)
                S1sN = sb2.tile([1, CQ], F32, tag="s1sN")
                nc.scalar.activation(S1sN[:], S1, CPY, scale=-1.0)
                nc.vector.tensor_scalar(S1S[:], S1, 1.0, None, op0=MUL)
                num = ps2.tile([128, C, QL], F32, tag="num")
                for c in range(C):
                    nc.tensor.matmul(num[:, c, :], mTu1[:, c, :], tqA[:, c, :],
                                     start=True, stop=False)
                    nc.tensor.matmul(num[:, c, :], mTu2[:, c, :], tqB[:, c, :],
                                     start=False, stop=False)
                    nc.tensor.matmul(num[:, c, :], mumrow[:, c, :],
                                     S1sN[:, QL * c:QL * (c + 1)],
                                     start=False, stop=True)
                sqA = sb2.tile([128, CQ], BF, tag="sqA")
                nc.vector.tensor_tensor(sqA[:], tqAf, tqAf, op=MUL)
                sqB = sb2.tile([25, CQ], BF, tag="sqB")
                nc.vector.tensor_tensor(sqB[:], tqBf, tqBf, op=MUL)
                sq1 = sb2.tile([1, CQ], BF, tag="sq1")
                nc.scalar.activation(sq1[:], S1, mybir.ActivationFunctionType.Square,
                                     scale=D ** -0.5)
                nc.tensor.matmul(S2, onesD[:, :], sqA[:], start=True, stop=False)
                nc.tensor.matmul(S2, onesD[0:25, :], sqB[:], start=False, stop=False)
                nc.tensor.matmul(S2, neg1[:], sq1[:], start=False, stop=True)
                nc.vector.tensor_tensor(nxS[:], num[:], ixn_b, op=MUL)
                nc.vector.tensor_scalar(yn2S[:], S2, 1.0, None, op0=MUL)
                p_t = p_tail(S2)
                if dbg:
                    nc.sync.dma_start(dbg_d["p1"][:], p_t[:])
                a_t = None

                for it in range(2):
                    gf = float(1 << it)      # tq is unnormalized: update adds (2^it s) hv
                    dsp = sb2.tile([128, C, QL], BF, tag="dsp")
                    if it == 0:
                        # softmax(0) = 1/C exactly
                        nc.vector.tensor_scalar(dsp[:].rearrange("p c q -> p (c q)"),
                                                p_t[:], 1.0 / C, None, op0=ADD)
                    else:
                        ea = sb2.tile([128, CQ], BF, tag="ea")
                        nc.scalar.activation(ea[:], a_t[:], EXP)
                        asum = sb2.tile([128, QL], BF, tag="asum")
                        nc.vector.tensor_reduce(asum[:], ea[:].rearrange("p (c q) -> p q c", c=C),
                                                axis=AX, op=ADD)
                        rs = sb2.tile([128, QL], BF, tag="rs")
                        nc.vector.reciprocal(rs[:], asum[:])
                        dd = sb2.tile([128, C, QL], BF, tag="dd")
                        nc.vector.tensor_tensor(
                            dd[:], ea[:].rearrange("p (c q) -> p c q", c=C),
                            rs[:].rearrange("p (a q) -> p a q", a=1).broadcast_to((128, C, QL)),
                            op=MUL)
                        nc.vector.tensor_tensor(dsp[:].rearrange("p c q -> p (c q)"),
                                                dd[:].rearrange("p c q -> p (c q)"), p_t[:], op=ADD)

                    # hv[d, (c,q)] in two D-chunks
                    hvA = ps2.tile([128, C, QL], F32, tag="hvA")
                    hvB = ps2.tile([26, C, QL], F32, tag="hvB")
                    for c in range(C):
                        nc.tensor.matmul(hvA[:, c, :], hat_m_bf[:, D * c:D * c + 128], dsp[:, c, :],
                                         start=True, stop=True)
                        nc.tensor.matmul(hvB[:, c, :], hat_m_bf[:, D * c + 128:D * c + 154], dsp[:, c, :],
                                         start=True, stop=True)
                    vAf = vA[:].rearrange("p c q -> p (c q)")
                    vBf = vB[:].rearrange("p c q -> p (c q)")
                    hvAf = hvA[:].rearrange("p c q -> p (c q)")
                    hvBf = hvB[0:25].rearrange("p c q -> p (c q)")
                    nc.scalar.copy(vA[:].rearrange("p c q -> p (c q)"), hvAf)
                    nc.vector.tensor_copy(vB[:].rearrange("p c q -> p (c q)"), hvBf)
                    sqhA = sb2.tile([128, CQ], BF, tag="sqhA")
                    nc.vector.tensor_tensor(sqhA[:], vAf, vAf, op=MUL)
                    sqhB = sb2.tile([25, CQ], BF, tag="sqhB")
                    nc.vector.tensor_tensor(sqhB[:], vBf, vBf, op=MUL)
                    rowsI = ps2.tile([1, 3 * CQ], F32, tag="rows")
                    n2 = rowsI[:, 0:CQ]
                    H1 = rowsI[:, CQ:2 * CQ]
                    X = rowsI[:, 2 * CQ:3 * CQ]
                    nc.tensor.matmul(n2, onesD[:, :], sqhA[:], start=True, stop=False)
                    nc.tensor.matmul(n2, onesD[0:25, :], sqhB[:], start=False, stop=True)
                    nc.tensor.matmul(H1, onesD[:, :], vAf, start=True, stop=False)
                    nc.tensor.matmul(H1, onesD[0:25, :], vBf, start=False, stop=True)
                    tqhA = sb2.tile([128, CQ], BF, tag="tqhA")
                    nc.vector.tensor_tensor(tqhA[:], tqAf, vAf, op=MUL)
                    tqhB = sb2.tile([25, CQ], BF, tag="tqhB")
                    nc.vector.tensor_tensor(tqhB[:], tqBf, vBf, op=MUL)
                    nc.tensor.matmul(X, onesD[:, :], tqhA[:], start=True, stop=False)
                    nc.tensor.matmul(X, onesD[0:25, :], tqhB[:], start=False, stop=True)
                    # H1sN = -H1 staged once (serves centering + row terms)
                    H1sN = sb2.tile([1, CQ], F32, tag="H1sN")
                    nc.scalar.activation(H1sN[:], H1, CPY, scale=-1.0)
                    # early row terms (pre-srow):
                    #   Ag = 2 gf (X - S1S*H1/D),  Bg = gf^2 (n2 - H1^2/D)
                    c1 = sb2.tile([1, CQ], F32, tag="c1")
                    nc.vector.tensor_tensor(c1[:], S1S[:], H1sN[:], op=MUL)   # = -S1*H1
                    c1s = sb2.tile([1, CQ], F32, tag="c1s")
                    nc.vector.tensor_scalar(c1s[:], c1[:], 2.0 * gf / D, None, op0=MUL)
                    X2 = sb2.tile([1, CQ], F32, tag="X2")
                    nc.vector.tensor_scalar(X2[:], X, 2.0 * gf, None, op0=MUL)
                    Ag = sb2.tile([1, CQ], F32, tag="Ag")
                    nc.vector.tensor_tensor(Ag[:], X2[:], c1s[:], op=ADD)
                    sqH = sb2.tile([1, CQ], F32, tag="sqH")
                    nc.vector.tensor_tensor(sqH[:], H1sN[:], H1sN[:], op=MUL)
                    sqHs = sb2.tile([1, CQ], F32, tag="sqHs")
                    nc.vector.tensor_scalar(sqHs[:], sqH[:], gf * gf / D, None, op0=MUL)
                    n2g = sb2.tile([1, CQ], F32, tag="n2g")
                    nc.vector.tensor_scalar(n2g[:], n2, gf * gf, None, op0=MUL)
                    Bg = sb2.tile([1, CQ], F32, tag="Bg")
                    nc.vector.tensor_tensor(Bg[:], n2g[:], sqHs[:], op=SUB)
                    # numh (centered via fused -mumd x H1) -> nxh = gf*numh*inv_xn
                    numh = ps2.tile([128, C, QL], F32, tag="num")
                    for c in range(C):
                        nc.tensor.matmul(numh[:, c, :], mTu1[:, c, :], vA[:, c, :],
                                         start=True, stop=False)
                        nc.tensor.matmul(numh[:, c, :], mTu2[:, c, :], vB[:, c, :],
                                         start=False, stop=False)
                        nc.tensor.matmul(numh[:, c, :], mumrow[:, c, :],
                                         H1sN[:, QL * c:QL * (c + 1)],
                                         start=False, stop=True)
                    nxh = sb2.tile([128, C, QL], F32, tag="nxh")
                    nc.vector.tensor_tensor(nxh[:], numh[:], ixn_b, op=MUL)
                    if it == 1:
                        nc.vector.tensor_scalar(nxh[:].rearrange("p c q -> p (c q)"),
                                                nxh[:].rearrange("p c q -> p (c q)"),
                                                gf, None, op0=MUL)
                    # mdv (uncentered m); pm = p*mdv early
                    mdv = ps2.tile([128, C, QL], F32, tag="mdv")
                    for c in range(C):
                        nc.tensor.matmul(mdv[:, c, :], mTu1[:, c, :], vA[:, c, :],
                                         start=True, stop=False)
                        nc.tensor.matmul(mdv[:, c, :], mTu2[:, c, :], vB[:, c, :],
                                         start=False, stop=True)
                    pm = sb2.tile([128, CQ], F32, tag="pm")
                    nc.vector.tensor_tensor(pm[:], mdv[:].rearrange("p c q -> p (c q)"), p_t[:], op=MUL)

                    # squash scale row: s = sqrt(n2)/(1+n2)
                    n2p1 = sb2.tile([1, CQ], F32, tag="n2p1")
                    nc.vector.tensor_scalar(n2p1[:], n2, 1.0, None, op0=ADD)
                    r1 = sb2.tile([1, CQ], F32, tag="r1")
                    nc.vector.reciprocal(r1[:], n2p1[:])
                    ln2 = sb2.tile([1, CQ], F32, tag="ln2")
                    nc.scalar.activation(ln2[:], n2, LN, bias=epsb[0:1, :])
                    sqn = sb2.tile([1, CQ], BF, tag="sqn")
                    nc.scalar.activation(sqn[:], ln2[:], EXP, scale=0.5)
                    srow = sb2.tile([1, CQ], BF, tag="srow")
                    nc.vector.tensor_tensor(srow[:], sqn[:], r1[:], op=MUL)
                    sB = ps2.tile([128, CQ], F32, tag="bcast")
                    nc.tensor.matmul(sB[:], ones1B[:], srow[:], start=True, stop=True)

                    # yn2S += s*Ag + s^2*Bg  (-> next pearson's denominator)
                    u1 = sb2.tile([1, CQ], F32, tag="u1")
                    nc.vector.tensor_tensor(u1[:], srow[:], Ag[:], op=MUL)
                    sq_s = sb2.tile([1, CQ], F32, tag="sq_s")
                    nc.vector.tensor_tensor(sq_s[:], srow[:], srow[:], op=MUL)
                    u2 = sb2.tile([1, CQ], F32, tag="u2")
                    nc.vector.tensor_tensor(u2[:], sq_s[:], Bg[:], op=MUL)
                    w = sb2.tile([1, CQ], F32, tag="w")
                    nc.vector.tensor_tensor(w[:], u1[:], u2[:], op=ADD)
                    nc.vector.tensor_tensor(yn2S[:], yn2S[:], w[:], op=ADD)
                    # S1S += gf * s * H1  (slack: needed next iteration only)
                    H1g = sb2.tile([1, CQ], F32, tag="H1g")
                    nc.vector.tensor_scalar(H1g[:], H1sN[:], -gf, None, op0=MUL)
                    sh = sb2.tile([1, CQ], F32, tag="sh")
                    nc.vector.tensor_tensor(sh[:], srow[:], H1g[:], op=MUL)
                    nc.vector.tensor_tensor(S1S[:], S1S[:], sh[:], op=ADD)
                    # nxS += s * nxh
                    nupd = sb2.tile([128, CQ], F32, tag="nupd")
                    nc.vector.tensor_tensor(nupd[:], sB[:], nxh[:].rearrange("p c q -> p (c q)"), op=MUL)
                    nc.vector.tensor_tensor(nxS[:].rearrange("p c q -> p (c q)"),
                                            nxS[:].rearrange("p c q -> p (c q)"), nupd[:], op=ADD)
                    # a += p * s * mdv
                    pms = sb2.tile([128, CQ], BF, tag="pms")
                    nc.vector.tensor_tensor(pms[:], pm[:], sB[:], op=MUL)
                    if it == 0:
                        a_t = pms
                    else:
                        a_new = sb2.tile([128, CQ], BF, tag="a")
                        nc.vector.tensor_tensor(a_new[:], a_t[:], pms[:], op=ADD)
                        a_t = a_new
                    # tq += (gf * s) * hv  (slack: feeds next iteration's X/S1 terms)
                    svA = sb2.tile([128, CQ], BF, tag="svA")
                    nc.vector.tensor_tensor(svA[:], vAf, sB[:], op=MUL)
                    svB = sb2.tile([25, CQ], BF, tag="svB")
                    nc.vector.tensor_tensor(svB[:], vBf, sB[0:25, :], op=MUL)
                    if it == 1:
                        nc.vector.tensor_scalar(svA[:], svA[:], 2.0, None, op0=MUL)
                        nc.vector.tensor_scalar(svB[:], svB[:], 2.0, None, op0=MUL)
                    nc.vector.tensor_tensor(tqAf, tqAf, svA[:], op=ADD)
                    nc.vector.tensor_tensor(tqBf, tqBf, svB[:], op=ADD)

                    p_t = p_tail(yn2S[:])
                    if dbg:
                        nc.sync.dma_start(dbg_d["a1" if it == 0 else "a2"][:], a_t[:])
                        nc.sync.dma_start(dbg_d["p2" if it == 0 else "p3"][:], p_t[:])

                # ---------------- final softmax ----------------
                ea = sb2.tile([128, CQ], BF, tag="ea")
                nc.scalar.activation(ea[:], a_t[:], EXP)
                asum = sb2.tile([128, QL], BF, tag="asum")
                nc.vector.tensor_reduce(asum[:], ea[:].rearrange("p (c q) -> p q c", c=C),
                                        axis=AX, op=ADD)
                rs = sb2.tile([128, QL], BF, tag="rs")
                nc.vector.reciprocal(rs[:], asum[:])
                dd = sb2.tile([128, C, QL], BF, tag="dd")
                nc.vector.tensor_tensor(
                    dd[:], ea[:].rearrange("p (c q) -> p c q", c=C),
                    rs[:].rearrange("p (a q) -> p a q", a=1).broadcast_to((128, C, QL)), op=MUL)
                dspF = sb2.tile([128, C, QL], BF, tag="dspbf")
                nc.vector.tensor_tensor(dspF[:].rearrange("p c q -> p (c q)"),
                                        dd[:].rearrange("p c q -> p (c q)"), p_t[:], op=ADD)

                # ---------------- final ----------------
                # per-c: matmul -> copy + square (Pool) + reduce (DVE), pipelined
                hvF = sb.tile([QL, CD], F32, tag="hvF")
                n2q = sb2.tile([QL, C], F32, tag="n2q")
                fsq = sb2.tile([QL, D], F32, tag="fsq")
                for c in range(C):
                    fps = ps2.tile([QL, D + 1], F32, tag=("hvA" if c % 2 == 0 else "mdv"))
                    nc.tensor.matmul(fps[:], dspF[:, c, :], hat_m_bf[:, D * c:D * c + 154],
                                     start=True, stop=True)
                    nc.vector.tensor_copy(hvF[:, D * c:D * (c + 1)], fps[:, 0:153])
                    nc.scalar.activation(fsq[:], fps[:, 0:153],
                                         mybir.ActivationFunctionType.Square,
                                         accum_out=n2q[:, c:c + 1])
                fp1 = sb2.tile([QL, C], F32, tag="fp1")
                nc.vector.tensor_scalar(fp1[:], n2q[:], 1.0, None, op0=ADD)
                fr1 = sb2.tile([QL, C], F32, tag="fr1")
                nc.vector.reciprocal(fr1[:], fp1[:])
                fln = sb2.tile([QL, C], F32, tag="fln")
                nc.scalar.activation(fln[:], n2q[:], mybir.ActivationFunctionType.Ln, bias=epsb[0:QL, :])
                fr2 = sb2.tile([QL, C], F32, tag="fr2")
                nc.scalar.activation(fr2[:], fln[:], mybir.ActivationFunctionType.Exp, scale=-0.5)
                fs1 = sb2.tile([QL, C], F32, tag="fs1")
                nc.vector.tensor_scalar(fs1[:], fr1[:], -1.0, 1.0, op0=MUL, op1=ADD)
                fs = sb2.tile([QL, C], F32, tag="fs")
                nc.vector.tensor_tensor(fs[:], fs1[:], fr2[:], op=MUL)
                # out = hvF * fs: c=0..2 on DVE -> sync DMA; c=3,4 on Act ->
                # scalar-queue DMA (same queue as producer: no cross sem)
                outT = sb.tile([QL, CD], F32, tag="outT")
                D3 = 3 * D
                nc.vector.tensor_tensor(
                    outT[:, 0:D3].rearrange("p (c d) -> p c d", c=3),
                    hvF[:, 0:D3].rearrange("p (c d) -> p c d", c=3),
                    fs[:, 0:3].rearrange("p (c a) -> p c a", a=1).broadcast_to((QL, 3, D)), op=MUL)
                nc.sync.dma_start(out_d[:, 0:D3], outT[:, 0:D3])
                for c in (3, 4):
                    nc.scalar.activation(outT[:, D * c:D * (c + 1)], hvF[:, D * c:D * (c + 1)],
                                         mybir.ActivationFunctionType.Copy, scale=fs[:, c:c + 1])
                nc.scalar.dma_start(out_d[:, D3:CD], outT[:, D3:CD])

    # All activations use only {Ln, Exp, Copy}, which live together in act
    # func set 6 (natural_log_exp_and_others). The default solver alternates
    # sets 0/5, inserting ~15 table reloads (~1.3us each); one load suffices.
    def _single_act_table_load():
        inst = mybir.InstLoadActFuncSet(
            name=nc.get_next_instruction_name(), ins=[], outs=[],
            act_func_set_id=6,
        )
        inst.engine = mybir.EngineType.Activation
        nc.register_instruction(inst)
        for blk in nc.main_func.blocks:
            for idx, bi in enumerate(blk.instructions):
                if isinstance(bi, mybir.InstActivation):
                    blk.instructions.insert(idx, inst)
                    return
        raise AssertionError("no activation found")

    nc.insert_act_table_loads = _single_act_table_load
    nc.compile()
    return nc


_CACHE = {}
LAST_EXEC_NS = None
LAST_RESULTS = None


def kernel(m, q, W, b):
    m = np.asarray(m, dtype=np.float32)
    q = np.asarray(q, dtype=np.float32)
    W = np.asarray(W, dtype=np.float32)
    b = np.asarray(b, dtype=np.float32)
    assert m.shape == (I, K) and q.shape == (NCORES * QL, K) and W.shape == (K, CD)

    with_bias = bool(np.any(b))
    dbg = bool(int(os.environ.get("KERNEL_DBG", "0")))
    key = ("v1", with_bias, str(DT), dbg)
    if key not in _CACHE:
        _CACHE[key] = build(with_bias, dbg)
    nc = _CACHE[key]

    import ml_dtypes
    BF = ml_dtypes.bfloat16

    def pre(x, n):  # [K, n] -> p-major [128, KC*n] bf16
        return np.ascontiguousarray(
            x.reshape(KC, 128, n).transpose(1, 0, 2).reshape(128, KC * n)).astype(BF)

    Wp = np.zeros((K, NPAD), dtype=np.float32)
    Wp[:, :CD] = W
    W_pre = pre(Wp, NPAD)
    mT_pre = pre(np.ascontiguousarray(m.T), I)
    eye = np.eye(128, dtype=np.float32)
    b2 = b.reshape(1, CD)

    in_maps = []
    for i in range(NCORES):
        qT_pre = pre(np.ascontiguousarray(q[QL * i:QL * (i + 1)].T), QL)
        in_maps.append({"mTp": mT_pre, "qTp": qT_pre, "Wpp": W_pre, "b": b2, "eye": eye})

    res = run_bass_kernel_spmd(nc, in_maps, list(range(NCORES)))
    global LAST_EXEC_NS, LAST_RESULTS
    LAST_EXEC_NS = res.exec_time_ns
    LAST_RESULTS = res.results
    out = np.concatenate([res.results[i]["out"] for i in range(NCORES)], axis=0)
    return out.astype(np.float32)


if __name__ == "__main__":
    rng = np.random.default_rng(0)
    m = rng.standard_normal((I, K)).astype(np.float32)
    q = rng.standard_normal((NCORES * QL, K)).astype(np.float32)
    W = (rng.standard_normal((K, CD)) * 0.02).astype(np.float32)
    b = np.zeros((CD,), dtype=np.float32)
    out = kernel(m=m, q=q, W=W, b=b)
    print("out", out.shape, out.dtype, np.abs(out).mean())



# revision 47
# speedup vs baseline: 1.0282x; 1.0018x over previous
"""DMR induction routing kernel for Trainium2 (Bass/Tile), 8-core data-parallel.

Problem: nn_DMRInduction. Full inputs:
  m [128, 768], q [256, 768], W [768, 765], b [765] -> out [256, 765] fp32.

Sharding: Q=256 split 8 ways (32 queries/core); m, W, b replicated.

Per-core layouts:
  - hat_m        [I=128, C*D=765]   (I on partitions)  - hv weights / final hv rhs
  - hmT aug      [D+1=154, I] per c (D on partitions)  - num/mdv weights;
      row 153 holds -mean_c(m) so the num matmul computes the centered
      correlation numerator directly (sum_d xm*tq = sum_d m*tq - mum*colsum).
  - tq, v        [D, C*Q=160] as two tiles [128,160] + [34,160]
      (tqB row 32 carries colsum for the augmented num matmul and the
       yn2 correction; vB rows 25..33 stay zero so mdv stays uncentered).
  - routing state a, p, dsp [I=128, C*Q=160].
  - final hat_v  [Q=32, C*D=765] -> squash -> contiguous DMA out.
"""
import os
import sys

for _p in ("/opt/trn_rl_repo", "/root/.axon_site/_ro/trn_rl_repo"):
    if os.path.isdir(_p) and _p not in sys.path:
        sys.path.insert(0, _p)

import numpy as np
import concourse.bass as bass
import concourse.bacc as bacc
import concourse.mybir as mybir
import concourse.tile as tile
from concourse.bass_utils import run_bass_kernel_spmd

F32 = mybir.dt.float32
# Matmul input dtype. float32 is exact (final scale-relative err ~2e-5);
# float32r uses the fast PE path (1 cyc/row at N>=256 vs 4) and cuts the
# projection phase ~14us, at ~2.5e-4 scale-relative output error. The
# rest of the kernel is dependency-latency-bound, so the dtype only
# affects the projection matmuls. Default to exact.
DT = getattr(mybir.dt, os.environ.get("KERNEL_MM_DT", "float32r"))

NCORES = 8
I = 128         # memory capsules
C = 5           # capsule classes
D = 153         # dim per capsule
CD = C * D      # 765
K = 768         # input dim
KC = K // 128   # 6 contraction chunks
QL = 32         # queries per core
CQ = C * QL     # 160
NPAD = 768      # W padded to 768 cols so fp32r matmuls stream N>=256
EPS = 1e-8
AX = mybir.AxisListType.X
MUL = mybir.AluOpType.mult
ADD = mybir.AluOpType.add
SUB = mybir.AluOpType.subtract


def build(with_bias: bool, dbg: bool = False):
    nc = bacc.Bacc("TRN2", target_bir_lowering=False, debug=False)

    BF = mybir.dt.bfloat16
    # inputs arrive host-prearranged p-major so each load is one contiguous
    # descriptor per partition: x_pre[p, k, n] = x[k*128+p, n]
    mT_d = nc.dram_tensor("mTp", [128, KC * I], BF, kind="ExternalInput")
    qT_d = nc.dram_tensor("qTp", [128, KC * QL], BF, kind="ExternalInput")
    W_d = nc.dram_tensor("Wpp", [128, KC * NPAD], BF, kind="ExternalInput")
    b_d = nc.dram_tensor("b", [1, CD], F32, kind="ExternalInput")
    eye_d = nc.dram_tensor("eye", [128, 128], F32, kind="ExternalInput")
    out_d = nc.dram_tensor("out", [QL, CD], F32, kind="ExternalOutput")
    dbg_d = {}
    if dbg:
        for nm, shp in [("hatm", [128, CD]), ("hatq", [QL, CD]), ("tqA0", [128, CQ]),
                        ("p1", [128, CQ]), ("a1", [128, CQ]), ("p2", [128, CQ]),
                        ("a2", [128, CQ]), ("p3", [128, CQ]), ("mTc1d", [128, C * 128]),
                        ("mTc2d", [25, C * 128]), ("tqB0", [25, CQ])]:
            dbg_d[nm] = nc.dram_tensor("dbg_" + nm, shp, F32, kind="ExternalOutput")

    with tile.TileContext(nc) as tc:
        with (
            nc.allow_low_precision("routing coefficients tolerate bf16"),
            tc.tile_pool(name="sb", bufs=1) as sb,
            tc.tile_pool(name="sb2", bufs=3) as sb2,
        ):
            # ---------------- loads ----------------
            # SP queue: mT then W per k-chunk (the projection stream);
            # Pool queue: qT, eye (needed later) so they don't delay W chunks.
            W_sb = sb.tile([128, KC, NPAD], BF, tag="W")
            mT_sb = sb.tile([128, KC, I], BF, tag="mT")
            qT_sb = sb.tile([128, KC, QL], BF, tag="qT")
            eye = sb.tile([128, 128], F32, tag="eye")
            nc.sync.dma_start(mT_sb[:], mT_d[:].rearrange("p (k n) -> p k n", k=KC))
            Wr = W_d[:].rearrange("p (k n) -> p k n", k=KC)
            for k in range(KC - 1):
                nc.sync.dma_start(W_sb[:, k, :], Wr[:, k, :])
            nc.sync.dma_start(qT_sb[:], qT_d[:].rearrange("p (k n) -> p k n", k=KC))
            nc.sync.dma_start(W_sb[:, KC - 1, :], Wr[:, KC - 1, :])
            nc.sync.dma_start(eye[:], eye_d[:])
            if with_bias:
                b_sb = sb.tile([1, CD], F32, tag="b")
                nc.sync.dma_start(b_sb[:], b_d[:])
            ones1 = sb.tile([1, 128], F32, tag="ones1")
            nc.vector.memset(ones1[:], 1.0)
            ones1B = sb.tile([1, 128], BF, tag="ones1B")
            nc.vector.memset(ones1B[:], 1.0)
            onesD = sb.tile([128, 1], BF, tag="onesD")
            nc.vector.memset(onesD[:], 1.0)
            epsb = sb.tile([128, 1], F32, tag="epsb")
            nc.vector.memset(epsb[:], EPS)

            # ---------------- projections (hat-major) ----------------
            hat_m_r = sb.tile([128, CD + 1], DT, tag="hatmr")  # col 765 zero (even-N pad)
            hat_q32 = sb.tile([QL, CD], F32, tag="hatq32")

            with tc.tile_pool(name="ps1", bufs=1, space="PSUM") as ps1, \
                 tc.tile_pool(name="pstp", bufs=4, space="PSUM") as pstp:
                psA = ps1.tile([128, 512], F32, tag="psA")
                psB = ps1.tile([128, 256], F32, tag="psB")
                for k in range(KC):
                    nc.tensor.matmul(psA[:], mT_sb[:, k, :], W_sb[:, k, 0:512],
                                     start=(k == 0), stop=(k == KC - 1 and not with_bias))
                    nc.tensor.matmul(psB[:], mT_sb[:, k, :], W_sb[:, k, 512:768],
                                     start=(k == 0), stop=(k == KC - 1 and not with_bias))
                if with_bias:
                    nc.tensor.matmul(psA[:], ones1[:], b_sb[:, 0:512], start=False, stop=True)
                    nc.tensor.matmul(psB[:, 0:253], ones1[:], b_sb[:, 512:765],
                                     start=False, stop=True)
                nc.scalar.copy(hat_m_r[:, 0:512], psA[:])
                nc.vector.tensor_copy(hat_m_r[:, 512:765], psB[:, 0:253])
                nc.vector.memset(hat_m_r[:, 765:766].bitcast(F32), 0.0)

                psC = ps1.tile([QL, 512], F32, tag="psC")
                psD = ps1.tile([QL, 256], F32, tag="psD")
                for k in range(KC):
                    nc.tensor.matmul(psC[:], qT_sb[:, k, :], W_sb[:, k, 0:512],
                                     start=(k == 0), stop=(k == KC - 1 and not with_bias))
                    nc.tensor.matmul(psD[:], qT_sb[:, k, :], W_sb[:, k, 512:768],
                                     start=(k == 0), stop=(k == KC - 1 and not with_bias))
                if with_bias:
                    onesq = sb.tile([1, QL], F32, tag="onesq")
                    nc.vector.memset(onesq[:], 1.0)
                    nc.tensor.matmul(psC[:], onesq[:], b_sb[:, 0:512],
                                     start=False, stop=True)
                    nc.tensor.matmul(psD[:, 0:253], onesq[:], b_sb[:, 512:765],
                                     start=False, stop=True)
                # NOTE: bias-for-q path writes b broadcast over q? must be b per column:
                # out[q, n] += 1*b[n] -> lhsT = onesq [1, QL], rhs = b [1, n] OK.
                nc.scalar.copy(hat_q32[:, 0:512], psC[:])
                nc.scalar.copy(hat_q32[:, 512:765], psD[:, 0:253])

                # ---------------- m stats ----------------
                # mum [128, C] on DVE (needed early for centering); xn2 via Act
                # square+accum per c (keeps DVE free for transpose copies)
                hm32 = hat_m_r[:, 0:765].bitcast(F32)
                mum = sb.tile([128, C], F32, tag="mum")
                nc.vector.tensor_reduce(mum[:], hm32.rearrange("p (c d) -> p c d", c=C),
                                        axis=AX, op=ADD)  # holds D*mean
                xn2 = sb.tile([128, C], F32, tag="xn2")
                sqm = sb.tile([128, CD], F32, tag="sqm")
                nc.gpsimd.tensor_tensor(sqm[:], hm32, hm32, op=MUL)
                # bf16 hat_m: moving operand for hv/final matmuls (pairs with
                # bf16 dsp); built on the otherwise-idle Pool engine
                hat_m_bf = sb.tile([128, CD + 1], BF, tag="hatmbf")
                nc.gpsimd.tensor_scalar(hat_m_bf[:], hat_m_r[:].bitcast(F32), 1.0, None, op0=MUL)
                nc.vector.tensor_reduce(xn2[:], sqm[:].rearrange("p (c d) -> p c d", c=C),
                                        axis=AX, op=ADD)
                # xn2 = sum(hm^2) - D*mum^2 ; inv_xn = 1/sqrt(xn2)
                mum2 = sb.tile([128, C], F32, tag="mum2")
                nc.vector.tensor_tensor(mum2[:], mum[:], mum[:], op=MUL)
                nc.vector.tensor_scalar(mum2[:], mum2[:], 1.0 / D, None, op0=MUL)
                nc.vector.tensor_tensor(xn2[:], xn2[:], mum2[:], op=SUB)
                lxn = sb.tile([128, C], F32, tag="lxn")
                nc.scalar.activation(lxn[:], xn2[:], mybir.ActivationFunctionType.Ln)
                inv_xn = sb.tile([128, C], F32, tag="invxn")
                nc.scalar.activation(inv_xn[:], lxn[:], mybir.ActivationFunctionType.Exp, scale=-0.5)

                # mumd = mum/D [128, C]: centering correction applied as
                # num_centered = num_raw - bcast(S1) * mumd (free-dim broadcast)
                mumd = sb.tile([128, C], F32, tag="mumd")
                nc.vector.tensor_scalar(mumd[:], mum[:], 1.0 / D, None, op0=MUL)

                # ---------------- transposes: hmT and tq ----------------
                mTu1 = sb.tile([128, C, 128], BF, tag="mTu1")   # rows d=0..127
                mTu2 = sb.tile([25, C, 128], BF, tag="mTu2")    # rows d=128..152
                tqA = sb.tile([128, C, QL], BF, tag="tqA")
                tqB = sb.tile([25, C, QL], BF, tag="tqB")
                vA = sb.tile([128, C, QL], BF, tag="vA")
                vB = sb.tile([25, C, QL], BF, tag="vB")

                for c in range(C):
                    t1 = pstp.tile([128, 128], F32, tag="tp")
                    nc.tensor.transpose(t1[:], hat_m_r[:, D * c:D * c + 128].bitcast(F32), eye[:])
                    nc.scalar.copy(mTu1[:, c, :], t1[:])
                    t2 = pstp.tile([25, 128], F32, tag="tp")
                    nc.tensor.transpose(t2[:], hat_m_r[:, D * c + 128:D * (c + 1)].bitcast(F32), eye[:])
                    nc.vector.tensor_copy(mTu2[:, c, :], t2[:])

                    t3 = pstp.tile([128, QL], F32, tag="tp")
                    nc.tensor.transpose(t3[:], hat_q32[:, D * c:D * c + 128], eye[0:QL, 0:QL])
                    nc.scalar.copy(tqA[:, c, :], t3[:])
                    t4 = pstp.tile([25, QL], F32, tag="tp")
                    nc.tensor.transpose(t4[:], hat_q32[:, D * c + 128:D * (c + 1)], eye[0:QL, 0:QL])
                    nc.vector.tensor_copy(tqB[:, c, :], t4[:])

            if dbg:
                nc.sync.dma_start(dbg_d["hatm"][:], hat_m_r[:, 0:765].bitcast(F32))
                nc.sync.dma_start(dbg_d["hatq"][:], hat_q32[:])
                nc.sync.dma_start(dbg_d["tqA0"][:], tqA[:].bitcast(F32).rearrange("p c q -> p (c q)"))
                nc.sync.dma_start(dbg_d["mTc1d"][:], mTc1[:].bitcast(F32).rearrange("p c q -> p (c q)"))
                nc.sync.dma_start(dbg_d["mTc2d"][:], mTc2[:].bitcast(F32).rearrange("p c q -> p (c q)"))
            # ---------------- routing ----------------
            with tc.tile_pool(name="ps2", bufs=1, space="PSUM") as ps2:
                LN = mybir.ActivationFunctionType.Ln
                EXP = mybir.ActivationFunctionType.Exp
                CPY = mybir.ActivationFunctionType.Copy
                ixn_b = inv_xn[:].rearrange("p (c a) -> p c a", a=1).broadcast_to((128, C, QL))
                tqAf = tqA[:].rearrange("p c q -> p (c q)")
                tqBf = tqB[:].rearrange("p c q -> p (c q)")
                neg1 = sb.tile([1, 1], BF, tag="neg1")
                nc.vector.memset(neg1[:], -1.0)
                # stage mumd as a [1, C*128] row: the centering correction
                # -mumd x colsum is fused into the num/numh psum groups
                tpm = ps2.tile([C, 128], F32, tag="rows")
                nc.tensor.transpose(tpm[:], mumd[:], eye[:])
                mumdT = sb.tile([C, 128], F32, tag="mumdT")
                nc.scalar.copy(mumdT[:], tpm[:])
                mumrow = sb.tile([1, C, 128], F32, tag="mumrow")
                nc.sync.dma_start(mumrow[:], mumdT[:, :])

                # persistent cross-iteration state (f32, incrementally updated)
                nxS = sb.tile([128, C, QL], F32, tag="nxS")   # (num - S1*mumd)*inv_xn
                yn2S = sb.tile([1, CQ], F32, tag="yn2S")      # sum tq^2 - S1^2/D
                S1S = sb.tile([1, CQ], F32, tag="S1S")        # colsum(tq)

                def p_tail(yn2_src):
                    """p = tanh(nxS * rsqrt(yn2_src))"""
                    lyn = sb2.tile([1, CQ], F32, tag="lyn")
                    nc.scalar.activation(lyn[:], yn2_src, LN)
                    inv_yn = sb2.tile([1, CQ], BF, tag="invyn")
                    nc.scalar.activation(inv_yn[:], lyn[:], EXP, scale=-0.5)
                    iyb = ps2.tile([128, CQ], F32, tag="bcast")
                    nc.tensor.matmul(iyb[:], ones1B[:], inv_yn[:], start=True, stop=True)
                    pp = sb2.tile([128, CQ], F32, tag="pp")
                    nc.vector.tensor_tensor(pp[:], nxS[:].rearrange("p c q -> p (c q)"),
                                            iyb[:], op=MUL)
                    # tanh(x) = 1 - 2/(1+exp(2x))
                    e2 = sb2.tile([128, CQ], F32, tag="e2")
                    nc.scalar.activation(e2[:], pp[:], EXP, scale=2.0)
                    den = sb2.tile([128, CQ], F32, tag="dent")
                    nc.vector.tensor_scalar(den[:], e2[:], 1.0, None, op0=ADD)
                    rr = sb2.tile([128, CQ], F32, tag="rr")
                    nc.vector.reciprocal(rr[:], den[:])
                    p_new = sb2.tile([128, CQ], BF, tag="p")
                    nc.vector.tensor_scalar(p_new[:], rr[:], -2.0, 1.0, op0=MUL, op1=ADD)
                    return p_new

                # ---- pearson #1 (full; initializes S1S / yn2S / nxS) ----
                rows1 = ps2.tile([1, 3 * CQ], F32, tag="rows")
                S1 = rows1[:, 0:CQ]
                S2 = rows1[:, CQ:2 * CQ]
                nc.tensor.matmul(S1, onesD[:, :], tqAf, start=True, stop=False)
                nc.tensor.matmul(S1, onesD[0:25, :], tqBf, start=False, stop=True)
                S1sN = sb2.tile([1, CQ], F32, tag="s1sN")
                nc.scalar.activation(S1sN[:], S1, CPY, scale=-1.0)
                nc.vector.tensor_scalar(S1S[:], S1, 1.0, None, op0=MUL)
                num = ps2.tile([128, C, QL], F32, tag="num")
                for c in range(C):
                    nc.tensor.matmul(num[:, c, :], mTu1[:, c, :], tqA[:, c, :],
                                     start=True, stop=False)
                    nc.tensor.matmul(num[:, c, :], mTu2[:, c, :], tqB[:, c, :],
                                     start=False, stop=False)
                    nc.tensor.matmul(num[:, c, :], mumrow[:, c, :],
                                     S1sN[:, QL * c:QL * (c + 1)],
                                     start=False, stop=True)
                sqA = sb2.tile([128, CQ], BF, tag="sqA")
                nc.vector.tensor_tensor(sqA[:], tqAf, tqAf, op=MUL)
                sqB = sb2.tile([25, CQ], BF, tag="sqB")
                nc.vector.tensor_tensor(sqB[:], tqBf, tqBf, op=MUL)
                sq1 = sb2.tile([1, CQ], BF, tag="sq1")
                nc.scalar.activation(sq1[:], S1, mybir.ActivationFunctionType.Square,
                                     scale=D ** -0.5)
                nc.tensor.matmul(S2, onesD[:, :], sqA[:], start=True, stop=False)
                nc.tensor.matmul(S2, onesD[0:25, :], sqB[:], start=False, stop=False)
                nc.tensor.matmul(S2, neg1[:], sq1[:], start=False, stop=True)
                nc.vector.tensor_tensor(nxS[:], num[:], ixn_b, op=MUL)
                nc.vector.tensor_scalar(yn2S[:], S2, 1.0, None, op0=MUL)
                p_t = p_tail(S2)
                if dbg:
                    nc.sync.dma_start(dbg_d["p1"][:], p_t[:])
                a_t = None

                for it in range(2):
                    gf = float(1 << it)      # tq is unnormalized: update adds (2^it s) hv
                    dsp = sb2.tile([128, C, QL], BF, tag="dsp")
                    if it == 0:
                        # softmax(0) = 1/C exactly
                        nc.vector.tensor_scalar(dsp[:].rearrange("p c q -> p (c q)"),
                                                p_t[:], 1.0 / C, None, op0=ADD)
                    else:
                        ea = sb2.tile([128, CQ], BF, tag="ea")
                        nc.scalar.activation(ea[:], a_t[:], EXP)
                        asum = sb2.tile([128, QL], BF, tag="asum")
                        nc.vector.tensor_reduce(asum[:], ea[:].rearrange("p (c q) -> p q c", c=C),
                                                axis=AX, op=ADD)
                        rs = sb2.tile([128, QL], BF, tag="rs")
                        nc.vector.reciprocal(rs[:], asum[:])
                        dd = sb2.tile([128, C, QL], BF, tag="dd")
                        nc.vector.tensor_tensor(
                            dd[:], ea[:].rearrange("p (c q) -> p c q", c=C),
                            rs[:].rearrange("p (a q) -> p a q", a=1).broadcast_to((128, C, QL)),
                            op=MUL)
                        nc.vector.tensor_tensor(dsp[:].rearrange("p c q -> p (c q)"),
                                                dd[:].rearrange("p c q -> p (c q)"), p_t[:], op=ADD)

                    # hv[d, (c,q)] in two D-chunks
                    hvA = ps2.tile([128, C, QL], F32, tag="hvA")
                    hvB = ps2.tile([26, C, QL], F32, tag="hvB")
                    for c in range(C):
                        nc.tensor.matmul(hvA[:, c, :], hat_m_bf[:, D * c:D * c + 128], dsp[:, c, :],
                                         start=True, stop=True)
                        nc.tensor.matmul(hvB[:, c, :], hat_m_bf[:, D * c + 128:D * c + 154], dsp[:, c, :],
                                         start=True, stop=True)
                    vAf = vA[:].rearrange("p c q -> p (c q)")
                    vBf = vB[:].rearrange("p c q -> p (c q)")
                    hvAf = hvA[:].rearrange("p c q -> p (c q)")
                    hvBf = hvB[0:25].rearrange("p c q -> p (c q)")
                    nc.scalar.copy(vA[:].rearrange("p c q -> p (c q)"), hvAf)
                    nc.vector.tensor_copy(vB[:].rearrange("p c q -> p (c q)"), hvBf)
                    sqhA = sb2.tile([128, CQ], BF, tag="sqhA")
                    nc.vector.tensor_tensor(sqhA[:], vAf, vAf, op=MUL)
                    sqhB = sb2.tile([25, CQ], BF, tag="sqhB")
                    nc.vector.tensor_tensor(sqhB[:], vBf, vBf, op=MUL)
                    rowsI = ps2.tile([1, 3 * CQ], F32, tag="rows")
                    n2 = rowsI[:, 0:CQ]
                    H1 = rowsI[:, CQ:2 * CQ]
                    X = rowsI[:, 2 * CQ:3 * CQ]
                    nc.tensor.matmul(n2, onesD[:, :], sqhA[:], start=True, stop=False)
                    nc.tensor.matmul(n2, onesD[0:25, :], sqhB[:], start=False, stop=True)
                    nc.tensor.matmul(H1, onesD[:, :], vAf, start=True, stop=False)
                    nc.tensor.matmul(H1, onesD[0:25, :], vBf, start=False, stop=True)
                    tqhA = sb2.tile([128, CQ], BF, tag="tqhA")
                    nc.vector.tensor_tensor(tqhA[:], tqAf, vAf, op=MUL)
                    tqhB = sb2.tile([25, CQ], BF, tag="tqhB")
                    nc.vector.tensor_tensor(tqhB[:], tqBf, vBf, op=MUL)
                    nc.tensor.matmul(X, onesD[:, :], tqhA[:], start=True, stop=False)
                    nc.tensor.matmul(X, onesD[0:25, :], tqhB[:], start=False, stop=True)
                    # H1sN = -H1 staged once (serves centering + row terms)
                    H1sN = sb2.tile([1, CQ], F32, tag="H1sN")
                    nc.scalar.activation(H1sN[:], H1, CPY, scale=-1.0)
                    # early row terms (pre-srow):
                    #   Ag = 2 gf (X - S1S*H1/D),  Bg = gf^2 (n2 - H1^2/D)
                    c1 = sb2.tile([1, CQ], F32, tag="c1")
                    nc.vector.tensor_tensor(c1[:], S1S[:], H1sN[:], op=MUL)   # = -S1*H1
                    c1s = sb2.tile([1, CQ], F32, tag="c1s")
                    nc.vector.tensor_scalar(c1s[:], c1[:], 2.0 * gf / D, None, op0=MUL)
                    X2 = sb2.tile([1, CQ], F32, tag="X2")
                    nc.vector.tensor_scalar(X2[:], X, 2.0 * gf, None, op0=MUL)
                    Ag = sb2.tile([1, CQ], F32, tag="Ag")
                    nc.vector.tensor_tensor(Ag[:], X2[:], c1s[:], op=ADD)
                    sqH = sb2.tile([1, CQ], F32, tag="sqH")
                    nc.vector.tensor_tensor(sqH[:], H1sN[:], H1sN[:], op=MUL)
                    sqHs = sb2.tile([1, CQ], F32, tag="sqHs")
                    nc.vector.tensor_scalar(sqHs[:], sqH[:], gf * gf / D, None, op0=MUL)
                    n2g = sb2.tile([1, CQ], F32, tag="n2g")
                    nc.vector.tensor_scalar(n2g[:], n2, gf * gf, None, op0=MUL)
                    Bg = sb2.tile([1, CQ], F32, tag="Bg")
                    nc.vector.tensor_tensor(Bg[:], n2g[:], sqHs[:], op=SUB)
                    # numh (centered via fused -mumd x H1) -> nxh = gf*numh*inv_xn
                    numh = ps2.tile([128, C, QL], F32, tag="num")
                    for c in range(C):
                        nc.tensor.matmul(numh[:, c, :], mTu1[:, c, :], vA[:, c, :],
                                         start=True, stop=False)
                        nc.tensor.matmul(numh[:, c, :], mTu2[:, c, :], vB[:, c, :],
                                         start=False, stop=False)
                        nc.tensor.matmul(numh[:, c, :], mumrow[:, c, :],
                                         H1sN[:, QL * c:QL * (c + 1)],
                                         start=False, stop=True)
                    nxh = sb2.tile([128, C, QL], F32, tag="nxh")
                    nc.vector.tensor_tensor(nxh[:], numh[:], ixn_b, op=MUL)
                    if it == 1:
                        nc.vector.tensor_scalar(nxh[:].rearrange("p c q -> p (c q)"),
                                                nxh[:].rearrange("p c q -> p (c q)"),
                                                gf, None, op0=MUL)
                    # mdv (uncentered m); pm = p*mdv early
                    mdv = ps2.tile([128, C, QL], F32, tag="mdv")
                    for c in range(C):
                        nc.tensor.matmul(mdv[:, c, :], mTu1[:, c, :], vA[:, c, :],
                                         start=True, stop=False)
                        nc.tensor.matmul(mdv[:, c, :], mTu2[:, c, :], vB[:, c, :],
                                         start=False, stop=True)
                    pm = sb2.tile([128, CQ], F32, tag="pm")
                    nc.vector.tensor_tensor(pm[:], mdv[:].rearrange("p c q -> p (c q)"), p_t[:], op=MUL)

                    # squash scale row: s = sqrt(n2)/(1+n2)
                    n2p1 = sb2.tile([1, CQ], F32, tag="n2p1")
                    nc.vector.tensor_scalar(n2p1[:], n2, 1.0, None, op0=ADD)
                    r1 = sb2.tile([1, CQ], F32, tag="r1")
                    nc.vector.reciprocal(r1[:], n2p1[:])
                    ln2 = sb2.tile([1, CQ], F32, tag="ln2")
                    nc.scalar.activation(ln2[:], n2, LN, bias=epsb[0:1, :])
                    sqn = sb2.tile([1, CQ], BF, tag="sqn")
                    nc.scalar.activation(sqn[:], ln2[:], EXP, scale=0.5)
                    srow = sb2.tile([1, CQ], BF, tag="srow")
                    nc.vector.tensor_tensor(srow[:], sqn[:], r1[:], op=MUL)
                    sB = ps2.tile([128, CQ], F32, tag="bcast")
                    nc.tensor.matmul(sB[:], ones1B[:], srow[:], start=True, stop=True)

                    # yn2S += s*Ag + s^2*Bg  (-> next pearson's denominator)
                    u1 = sb2.tile([1, CQ], F32, tag="u1")
                    nc.vector.tensor_tensor(u1[:], srow[:], Ag[:], op=MUL)
                    sq_s = sb2.tile([1, CQ], F32, tag="sq_s")
                    nc.vector.tensor_tensor(sq_s[:], srow[:], srow[:], op=MUL)
                    u2 = sb2.tile([1, CQ], F32, tag="u2")
                    nc.vector.tensor_tensor(u2[:], sq_s[:], Bg[:], op=MUL)
                    w = sb2.tile([1, CQ], F32, tag="w")
                    nc.vector.tensor_tensor(w[:], u1[:], u2[:], op=ADD)
                    nc.vector.tensor_tensor(yn2S[:], yn2S[:], w[:], op=ADD)
                    # S1S += gf * s * H1  (slack: needed next iteration only)
                    H1g = sb2.tile([1, CQ], F32, tag="H1g")
                    nc.vector.tensor_scalar(H1g[:], H1sN[:], -gf, None, op0=MUL)
                    sh = sb2.tile([1, CQ], F32, tag="sh")
                    nc.vector.tensor_tensor(sh[:], srow[:], H1g[:], op=MUL)
                    nc.vector.tensor_tensor(S1S[:], S1S[:], sh[:], op=ADD)
                    # nxS += s * nxh
                    nupd = sb2.tile([128, CQ], F32, tag="nupd")
                    nc.vector.tensor_tensor(nupd[:], sB[:], nxh[:].rearrange("p c q -> p (c q)"), op=MUL)
                    nc.vector.tensor_tensor(nxS[:].rearrange("p c q -> p (c q)"),
                                            nxS[:].rearrange("p c q -> p (c q)"), nupd[:], op=ADD)
                    # a += p * s * mdv
                    pms = sb2.tile([128, CQ], BF, tag="pms")
                    nc.vector.tensor_tensor(pms[:], pm[:], sB[:], op=MUL)
                    if it == 0:
                        a_t = pms
                    else:
                        a_new = sb2.tile([128, CQ], BF, tag="a")
                        nc.vector.tensor_tensor(a_new[:], a_t[:], pms[:], op=ADD)
                        a_t = a_new
                    # tq += (gf * s) * hv  (slack: feeds next iteration's X/S1 terms)
                    svA = sb2.tile([128, CQ], BF, tag="svA")
                    nc.vector.tensor_tensor(svA[:], vAf, sB[:], op=MUL)
                    svB = sb2.tile([25, CQ], BF, tag="svB")
                    nc.vector.tensor_tensor(svB[:], vBf, sB[0:25, :], op=MUL)
                    if it == 1:
                        nc.vector.tensor_scalar(svA[:], svA[:], 2.0, None, op0=MUL)
                        nc.vector.tensor_scalar(svB[:], svB[:], 2.0, None, op0=MUL)
                    nc.vector.tensor_tensor(tqAf, tqAf, svA[:], op=ADD)
                    nc.vector.tensor_tensor(tqBf, tqBf, svB[:], op=ADD)

                    p_t = p_tail(yn2S[:])
                    if dbg:
                        nc.sync.dma_start(dbg_d["a1" if it == 0 else "a2"][:], a_t[:])
                        nc.sync.dma_start(dbg_d["p2" if it == 0 else "p3"][:], p_t[:])

                # ---------------- final softmax ----------------
                ea = sb2.tile([128, CQ], BF, tag="ea")
                nc.scalar.activation(ea[:], a_t[:], EXP)
                asum = sb2.tile([128, QL], BF, tag="asum")
                nc.vector.tensor_reduce(asum[:], ea[:].rearrange("p (c q) -> p q c", c=C),
                                        axis=AX, op=ADD)
                rs = sb2.tile([128, QL], BF, tag="rs")
                nc.vector.reciprocal(rs[:], asum[:])
                dd = sb2.tile([128, C, QL], BF, tag="dd")
                nc.vector.tensor_tensor(
                    dd[:], ea[:].rearrange("p (c q) -> p c q", c=C),
                    rs[:].rearrange("p (a q) -> p a q", a=1).broadcast_to((128, C, QL)), op=MUL)
                dspF = sb2.tile([128, C, QL], BF, tag="dspbf")
                nc.vector.tensor_tensor(dspF[:].rearrange("p c q -> p (c q)"),
                                        dd[:].rearrange("p c q -> p (c q)"), p_t[:], op=ADD)

                # ---------------- final ----------------
                # per-c: matmul -> copy + square (Pool) + reduce (DVE), pipelined
                hvF = sb.tile([QL, CD], F32, tag="hvF")
                n2q = sb2.tile([QL, C], F32, tag="n2q")
                fsq = sb2.tile([QL, D], F32, tag="fsq")
                for c in range(C):
                    fps = ps2.tile([QL, D + 1], F32, tag=("hvA" if c % 2 == 0 else "mdv"))
                    nc.tensor.matmul(fps[:], dspF[:, c, :], hat_m_bf[:, D * c:D * c + 154],
                                     start=True, stop=True)
                    nc.vector.tensor_copy(hvF[:, D * c:D * (c + 1)], fps[:, 0:153])
                    nc.scalar.activation(fsq[:], fps[:, 0:153],
                                         mybir.ActivationFunctionType.Square,
                                         accum_out=n2q[:, c:c + 1])
                fp1 = sb2.tile([QL, C], F32, tag="fp1")
                nc.vector.tensor_scalar(fp1[:], n2q[:], 1.0, None, op0=ADD)
                fr1 = sb2.tile([QL, C], F32, tag="fr1")
                nc.vector.reciprocal(fr1[:], fp1[:])
                fln = sb2.tile([QL, C], F32, tag="fln")
                nc.scalar.activation(fln[:], n2q[:], mybir.ActivationFunctionType.Ln, bias=epsb[0:QL, :])
                fr2 = sb2.tile([QL, C], F32, tag="fr2")
                nc.scalar.activation(fr2[:], fln[:], mybir.ActivationFunctionType.Exp, scale=-0.5)
                fs1 = sb2.tile([QL, C], F32, tag="fs1")
                nc.vector.tensor_scalar(fs1[:], fr1[:], -1.0, 1.0, op0=MUL, op1=ADD)
                fs = sb2.tile([QL, C], F32, tag="fs")
                nc.vector.tensor_tensor(fs[:], fs1[:], fr2[:], op=MUL)
                # out = hvF * fs: c=0..2 on DVE -> sync DMA; c=3,4 on Act ->
                # scalar-queue DMA (same queue as producer: no cross sem)
                outT = sb.tile([QL, CD], F32, tag="outT")
                D3 = 3 * D
                nc.vector.tensor_tensor(
                    outT[:, 0:D3].rearrange("p (c d) -> p c d", c=3),
                    hvF[:, 0:D3].rearrange("p (c d) -> p c d", c=3),
                    fs[:, 0:3].rearrange("p (c a) -> p c a", a=1).broadcast_to((QL, 3, D)), op=MUL)
                nc.sync.dma_start(out_d[:, 0:D3], outT[:, 0:D3])
                for c in (3, 4):
                    nc.scalar.activation(outT[:, D * c:D * (c + 1)], hvF[:, D * c:D * (c + 1)],
                                         mybir.ActivationFunctionType.Copy, scale=fs[:, c:c + 1])
                nc.scalar.dma_start(out_d[:, D3:CD], outT[:, D3:CD])

    # All activations use only {Ln, Exp, Copy}, which live together in act
    # func set 6 (natural_log_exp_and_others). The default solver alternates
    # sets 0/5, inserting ~15 table reloads (~1.3us each); one load suffices.
    def _single_act_table_load():
        inst = mybir.InstLoadActFuncSet(
            name=nc.get_next_instruction_name(), ins=[], outs=[],
            act_func_set_id=6,
        )
        inst.engine = mybir.EngineType.Activation
        nc.register_instruction(inst)
        for blk in nc.main_func.blocks:
            for idx, bi in enumerate(blk.instructions):
                if isinstance(bi, mybir.InstActivation):
                    blk.instructions.insert(idx, inst)
                    return
        raise AssertionError("no activation found")

    nc.insert_act_table_loads = _single_act_table_load
    nc.compile()
    return nc


_CACHE = {}
LAST_EXEC_NS = None
LAST_RESULTS = None


def kernel(m, q, W, b):
    m = np.asarray(m, dtype=np.float32)
    q = np.asarray(q, dtype=np.float32)
    W = np.asarray(W, dtype=np.float32)
    b = np.asarray(b, dtype=np.float32)
    assert m.shape == (I, K) and q.shape == (NCORES * QL, K) and W.shape == (K, CD)

    with_bias = bool(np.any(b))
    dbg = bool(int(os.environ.get("KERNEL_DBG", "0")))
    key = ("v1", with_bias, str(DT), dbg)
    if key not in _CACHE:
        _CACHE[key] = build(with_bias, dbg)
    nc = _CACHE[key]

    import ml_dtypes
    BF = ml_dtypes.bfloat16

    def pre(x, n):  # [K, n] -> p-major [128, KC*n] bf16
        return np.ascontiguousarray(
            x.reshape(KC, 128, n).transpose(1, 0, 2).reshape(128, KC * n)).astype(BF)

    Wp = np.zeros((K, NPAD), dtype=np.float32)
    Wp[:, :CD] = W
    W_pre = pre(Wp, NPAD)
    mT_pre = pre(np.ascontiguousarray(m.T), I)
    eye = np.eye(128, dtype=np.float32)
    b2 = b.reshape(1, CD)

    in_maps = []
    for i in range(NCORES):
        qT_pre = pre(np.ascontiguousarray(q[QL * i:QL * (i + 1)].T), QL)
        in_maps.append({"mTp": mT_pre, "qTp": qT_pre, "Wpp": W_pre, "b": b2, "eye": eye})

    res = run_bass_kernel_spmd(nc, in_maps, list(range(NCORES)))
    global LAST_EXEC_NS, LAST_RESULTS
    LAST_EXEC_NS = res.exec_time_ns
    LAST_RESULTS = res.results
    out = np.concatenate([res.results[i]["out"] for i in range(NCORES)], axis=0)
    return out.astype(np.float32)


if __name__ == "__main__":
    rng = np.random.default_rng(0)
    m = rng.standard_normal((I, K)).astype(np.float32)
    q = rng.standard_normal((NCORES * QL, K)).astype(np.float32)
    W = (rng.standard_normal((K, CD)) * 0.02).astype(np.float32)
    b = np.zeros((CD,), dtype=np.float32)
    out = kernel(m=m, q=q, W=W, b=b)
    print("out", out.shape, out.dtype, np.abs(out).mean())

